# revision 16
# baseline (speedup 1.0000x reference)
"""Trainium2 Bass kernel for nn_MoEStage (MoE routing stage).

Data-parallel over batch B=32 across 8 NeuronCores (4 batches/core).
All params replicated and pre-folded on host:
  - LayerNorm affine (ln_g, ln_b) folded into Wr1/We1h weights+biases
  - stage_idx/expert_idx gathers folded into weight matrices
  - Wf @ We1f collapsed into a single [64, 64] per-expert matrix
  - alpha folded into We2/be2

Device layout: token-major tiles [128 tokens, D] for LN/softmax/top-k,
feature-major [D, tokens] (via PE transposes) for all matmuls.
"""
import sys
import functools

sys.path.insert(0, "/opt/trn_rl_repo")

import numpy as np
import ml_dtypes

import concourse.bacc as bacc
import concourse.mybir as mybir
import concourse.tile as tile
from concourse.bass import broadcast_tensor_aps
from concourse.bass_utils import run_bass_kernel_spmd

F32 = mybir.dt.float32
F32R = mybir.dt.float32r
BF16 = mybir.dt.bfloat16

def r32(ap):
    return ap.bitcast(F32R)

AF = mybir.ActivationFunctionType
ALU = mybir.AluOpType
AX = mybir.AxisListType

B, T, DM, DFE, DH, DRH = 32, 2048, 128, 64, 64, 64
E, NF, NSF, TOPK = 8, 16, 64, 2
LN_EPS = 1e-5
NCORES = 8
BC = B // NCORES          # batches per core
NTOK = BC * T             # 8192 tokens per core
NT = NTOK // 128          # 64 token-tiles of 128
GSZ = 4                   # tiles per group
NG = NT // GSZ            # 16 groups of 512 tokens
GW = GSZ * 128            # 512 tokens per group
NPAIR = 4                 # expert pairs


def build_kernel(ntok=NTOK):
    nt = ntok // 128
    ng = nt // GSZ
    nc = bacc.Bacc("TRN2", target_bir_lowering=False, debug=False,
                   num_devices=NCORES)

    din = {}
    R32_INPUTS = set()
    BF_INPUTS = {"w_cmb", "w_e1h", "w_e2", "b_e2", "sel"}
    def inp(name, shape):
        dt = BF16 if name in BF_INPUTS else F32
        din[name] = nc.dram_tensor(name, list(shape), dt, kind="ExternalInput")
        return din[name]

    hidden = inp("hidden", [ntok, DM])
    feat = inp("feat", [ntok, NSF])
    valid = inp("valid", [128, nt])
    w_r1h = inp("w_r1h", [DM, DRH])          # lhsT K=128 -> M=64
    w_r1f = inp("w_r1f", [NSF, DRH])         # lhsT K=64 -> M=64
    b_r1 = inp("b_r1", [2 * DRH, 1])   # duplicated for col-packed pairs
    w_r2a = inp("w_r2a", [2 * DRH, 2 * E])   # block-diag [Wr2 0; 0 Wr2]
    br2exp = inp("br2exp", [128, E])
    w_cmb = inp("w_cmb", [NSF, NPAIR * 128])   # per pair: [64, 128]
    w_e1h = inp("w_e1h", [DM, NPAIR * 128])    # per pair: [128, 128]
    b_e1 = inp("b_e1", [128, NPAIR])
    w_e2 = inp("w_e2", [128, NPAIR * 128])     # chunk c: [K=128, M=128]
    b_e2 = inp("b_e2", [E, DM])
    sel = inp("sel", [E, NPAIR * 128])
    ident = inp("ident", [128, 128])

    next_h = nc.dram_tensor("next_h", [ntok, DM], F32, kind="ExternalOutput")
    gates_o = nc.dram_tensor("gates_o", [ntok, E], F32, kind="ExternalOutput")
    probs_o = nc.dram_tensor("probs_o", [ntok, E], F32, kind="ExternalOutput")

    with tile.TileContext(nc) as tc:
        with (
            tc.tile_pool(name="wp", bufs=1) as wp,
            tc.tile_pool(name="big", bufs=1) as big,
            tc.tile_pool(name="wk", bufs=3) as wk,
            tc.tile_pool(name="grp", bufs=2) as grp,
            tc.tile_pool(name="ps", bufs=2, space="PSUM") as ps,
            tc.tile_pool(name="ps1", bufs=1, space="PSUM") as ps1,
        ):
            # ---- load weights/constants ----
            wt = {}
            for name, h in din.items():
                if name in ("hidden", "feat"):
                    continue
                t = wp.tile(list(h.shape), h.dtype, tag=f"w_{name}")
                nc.sync.dma_start(t[:], h[:])
                wt[name] = t

            # ---- load activations (token-major, hidden padded for bn_stats) ----
            hid_tm = big.tile([128, nt, 132], F32)
            nc.sync.dma_start(hid_tm[:, :, 0:128],
                              hidden.ap().rearrange("(n p) d -> p n d", p=128))
            feat_tm = big.tile([128, nt, NSF], F32)
            nc.sync.dma_start(feat_tm[:],
                              feat.ap().rearrange("(n p) d -> p n d", p=128))

            # ---- LayerNorm stats via per-tile bn_stats ----
            bn = big.tile([128, nt, 8], F32)
            for n in range(nt):
                nc.vector.bn_stats(bn[:, n, 0:6], hid_tm[:, n, 0:128])
            me, ve = bn[:, :, 1], bn[:, :, 2]
            mo, vo = bn[:, :, 4], bn[:, :, 5]
            mu = big.tile([128, nt], F32)
            nc.vector.tensor_tensor(mu[:], me, mo, ALU.add)
            nc.vector.tensor_scalar_mul(mu[:], mu[:], 0.5)
            dmn = big.tile([128, nt], F32)
            nc.vector.tensor_tensor(dmn[:], me, mo, ALU.subtract)
            nc.vector.tensor_tensor(dmn[:], dmn[:], dmn[:], ALU.mult)
            var = big.tile([128, nt], F32)
            nc.vector.tensor_tensor(var[:], ve, vo, ALU.add)
            nc.vector.scalar_tensor_tensor(var[:], var[:], 1.0 / DM, dmn[:],
                                           ALU.mult, ALU.bypass)
            nc.vector.scalar_tensor_tensor(var[:], dmn[:], 0.25, var[:],
                                           ALU.mult, ALU.add)
            rstd = big.tile([128, nt], F32)
            epsb = wp.tile([128, 1], F32, tag="epsb")
            nc.vector.memset(epsb[:], LN_EPS)
            nc.scalar.activation(rstd[:], var[:], AF.Ln, bias=epsb[:])
            nc.scalar.activation(rstd[:], rstd[:], AF.Exp, scale=-0.5)

            identb = wp.tile([128, 128], BF16, tag="identb")
            nc.gpsimd.tensor_copy(identb[:], wt["ident"][:])

            h_fm = big.tile([128, ntok], F32)
            f_fm = big.tile([NSF, ntok], F32)
            logit_sb = big.tile([128, nt, E], F32)
            Eexp = big.tile([128, nt, E], F32)
            probs = big.tile([128, nt, E], F32)
            msk = big.tile([128, nt, E], F32)
            G0 = big.tile([128, nt, E], F32)
            gates = big.tile([128, nt, E], F32)
            gates_bf = big.tile([128, nt, E], BF16)
            g_fm = big.tile([E, ntok], BF16)
            S = big.tile([128, nt], F32)
            Sr = big.tile([128, nt], F32)
            m1 = big.tile([128, nt], F32)
            m2 = big.tile([128, nt], F32)
            dsum = big.tile([128, nt], F32)
            dr = big.tile([128, nt], F32)

            def bcol(t, sl):  # [128, cw] -> broadcastable [128, cw, E]
                return t[:, sl].rearrange("p (n o) -> p n o", o=1)

            NCH = 4                       # chunks
            CG = ng // NCH                # groups per chunk (4)
            CT = CG * GSZ                 # tiles per chunk (16)
            probs_od = probs_o.ap().rearrange("(n p) e -> p n e", p=128)
            gates_od = gates_o.ap().rearrange("(n p) e -> p n e", p=128)
            next_od = next_h.ap().rearrange("(n p) d -> p n d", p=128)

            for c in range(NCH):
                tsl = slice(CT * c, CT * (c + 1))          # tile slice
                # ---- normalize + transpose to feature-major ----
                for g in range(CG * c, CG * (c + 1)):
                    hps = ps.tile([128, GW], F32, tag="psA")
                    fps = ps.tile([NSF, GW], F32, tag="psB")
                    for s in range(GSZ):
                        n = GSZ * g + s
                        xn = wk.tile([128, 128], F32, tag="xn")
                        nc.gpsimd.tensor_scalar(xn[:], hid_tm[:, n, 0:128],
                                                mu[:, n:n + 1], rstd[:, n:n + 1],
                                                ALU.subtract, ALU.mult)
                        nc.tensor.transpose(hps[:, 128 * s:128 * (s + 1)],
                                            xn[:], wt["ident"][:])
                        nc.tensor.transpose(fps[:, 128 * s:128 * (s + 1)],
                                            feat_tm[:, n, :], wt["ident"][:])
                    gsl = slice(GW * g, GW * (g + 1))
                    nc.scalar.copy(h_fm[:, gsl], hps[:])
                    nc.scalar.copy(f_fm[:, gsl], fps[:])

                # ---- router MLP (two groups packed per PE pass) ----
                for g2 in range(CG * c // 2, CG * (c + 1) // 2):
                    sl0 = slice(GW * 2 * g2, GW * (2 * g2 + 1))
                    sl1 = slice(GW * (2 * g2 + 1), GW * (2 * g2 + 2))
                    r1p = ps1.tile([128, 2, GW], F32, tag="psC")
                    nc.tensor.matmul(r1p[0:DRH, 0, :], wt["w_r1h"][:],
                                     h_fm[:, sl0], tile_position=(0, 0),
                                     start=True, stop=False)
                    nc.tensor.matmul(r1p[DRH:2 * DRH, 1, :], wt["w_r1h"][:],
                                     h_fm[:, sl1], tile_position=(0, 64),
                                     start=True, stop=False)
                    nc.tensor.matmul(r1p[0:DRH, 0, :], wt["w_r1f"][:],
                                     f_fm[:, sl0], tile_position=(0, 0),
                                     start=False, stop=True)
                    nc.tensor.matmul(r1p[DRH:2 * DRH, 1, :], wt["w_r1f"][:],
                                     f_fm[:, sl1], tile_position=(0, 64),
                                     start=False, stop=True)
                    r1sb = grp.tile([128, GW], F32, tag="r1sb")
                    nc.scalar.activation(r1sb[0:DRH, :], r1p[0:DRH, 0, :],
                                         AF.Relu, bias=wt["b_r1"][0:DRH, :])
                    nc.scalar.activation(r1sb[DRH:128, :], r1p[DRH:128, 1, :],
                                         AF.Relu, bias=wt["b_r1"][DRH:128, :])
                    logit_ps = ps1.tile([128, GSZ, 2, E], F32, tag="psD")
                    for s in range(GSZ):
                        nc.tensor.matmul(logit_ps[:, s, :, :],
                                         r1sb[:, 128 * s:128 * (s + 1)],
                                         wt["w_r2a"][:], start=True, stop=True)
                    dst = logit_sb[:, GSZ * 2 * g2:GSZ * (2 * g2 + 2), :]
                    nc.scalar.copy(dst.rearrange("p (h s) e -> p s h e", h=2),
                                   logit_ps[:])

                # ---- softmax (logits are O(1); no max-sub) + top-2 ----
                nc.scalar.activation(Eexp[:, tsl, :], logit_sb[:, tsl, :],
                                     AF.Exp)
                a, b_ = broadcast_tensor_aps(
                    Eexp[:, tsl, :],
                    wt["br2exp"][:].rearrange("p (n e) -> p n e", n=1))
                nc.vector.tensor_tensor(Eexp[:, tsl, :], a, b_, ALU.mult)
                nc.vector.reduce_sum(S[:, tsl], Eexp[:, tsl, :], axis=AX.X)
                nc.vector.reciprocal(Sr[:, tsl], S[:, tsl])
                a, b_ = broadcast_tensor_aps(Eexp[:, tsl, :], bcol(Sr, tsl))
                nc.vector.tensor_tensor(probs[:, tsl, :], a, b_, ALU.mult)
                nc.sync.dma_start(probs_od[:, tsl, :], probs[:, tsl, :])
                nc.vector.reduce_max(m1[:, tsl], Eexp[:, tsl, :], axis=AX.X)
                a, b_ = broadcast_tensor_aps(Eexp[:, tsl, :], bcol(m1, tsl))
                nc.vector.tensor_tensor(msk[:, tsl, :], a, b_, ALU.is_lt)
                nc.vector.tensor_tensor(G0[:, tsl, :], Eexp[:, tsl, :],
                                        msk[:, tsl, :], ALU.mult)
                nc.vector.reduce_max(m2[:, tsl], G0[:, tsl, :], axis=AX.X)
                a, b_ = broadcast_tensor_aps(Eexp[:, tsl, :], bcol(m2, tsl))
                nc.vector.tensor_tensor(msk[:, tsl, :], a, b_, ALU.is_ge)
                nc.vector.tensor_tensor(G0[:, tsl, :], Eexp[:, tsl, :],
                                        msk[:, tsl, :], ALU.mult)
                nc.vector.reduce_sum(dsum[:, tsl], G0[:, tsl, :], axis=AX.X)
                nc.vector.reciprocal(dr[:, tsl], dsum[:, tsl])
                nc.vector.tensor_tensor(dr[:, tsl], dr[:, tsl],
                                        wt["valid"][:, tsl], ALU.mult)
                a, b_ = broadcast_tensor_aps(G0[:, tsl, :], bcol(dr, tsl))
                nc.vector.tensor_tensor(gates[:, tsl, :], a, b_, ALU.mult)
                nc.sync.dma_start(gates_od[:, tsl, :], gates[:, tsl, :])

                # ---- gates to feature-major (bf16, combine-only) ----
                nc.gpsimd.tensor_copy(gates_bf[:, tsl, :], gates[:, tsl, :])
                for g in range(CG * c, CG * (c + 1)):
                    gps = ps1.tile([E, GW], BF16, tag="psD")
                    for s in range(GSZ):
                        n = GSZ * g + s
                        nc.tensor.transpose(gps[:, 128 * s:128 * (s + 1)],
                                            gates_bf[:, n, :], identb[:])
                    nc.scalar.copy(g_fm[:, GW * g:GW * (g + 1)], gps[:])

                # ---- experts (dense, pairs of 2) + combine + residual ----
                for g in range(CG * c, CG * (c + 1)):
                    gsl = slice(GW * g, GW * (g + 1))
                    h_fr = grp.tile([128, GW], BF16, tag="h_fr")
                    nc.gpsimd.tensor_copy(h_fr[:], h_fm[:, gsl])
                    f_fr = grp.tile([NSF, GW], BF16, tag="f_fr")
                    nc.gpsimd.tensor_copy(f_fr[:], f_fm[:, gsl])
                    moe = ps1.tile([128, GW], F32, tag="psCm")
                    for p in range(NPAIR):
                        psl_ = slice(128 * p, 128 * (p + 1))
                        h1p = ps.tile([128, GW], F32, tag="psA")
                        nc.tensor.matmul(h1p[:], wt["w_e1h"][:, psl_],
                                         h_fr[:], start=True, stop=False)
                        nc.tensor.matmul(h1p[:], wt["w_cmb"][:, psl_],
                                         f_fr[:], start=False, stop=True)
                        h1 = grp.tile([128, GW], BF16, tag="h1")
                        nc.scalar.activation(h1[:], h1p[:], AF.Gelu,
                                             bias=wt["b_e1"][:, p:p + 1])
                        gb = ps.tile([128, GW], F32, tag="psB")
                        nc.tensor.matmul(gb[:], wt["sel"][:, psl_],
                                         g_fm[:, gsl], start=True, stop=True)
                        h1s = grp.tile([128, GW], BF16, tag="h1s")
                        nc.vector.tensor_tensor(h1s[:], h1[:], gb[:], ALU.mult)
                        nc.tensor.matmul(moe[:], wt["w_e2"][:, psl_], h1s[:],
                                         start=(p == 0), stop=False)
                    nc.tensor.matmul(moe[:], wt["b_e2"][:], g_fm[:, gsl],
                                     start=False, stop=True)
                    moe_sb = grp.tile([128, GW], BF16, tag="moe_sb")
                    nc.scalar.copy(moe_sb[:], moe[:])
                    moeT = grp.tile([128, GSZ, 128], BF16, tag="moeT")
                    nc.sync.dma_start_transpose(moeT[:], moe_sb[:])
                    nxt = grp.tile([128, GSZ, 128], F32, tag="nxt")
                    nc.vector.tensor_tensor(
                        nxt[:], hid_tm[:, GSZ * g:GSZ * (g + 1), 0:128],
                        moeT[:], ALU.add)
                    nc.sync.dma_start(next_od[:, GSZ * g:GSZ * (g + 1), :],
                                      nxt[:])

    nc.compile()
    return nc


@functools.cache
def _built():
    return build_kernel()


def _fold_weights(inputs):
    """Host-side folding of all parameters into device weight tensors."""
    f32 = np.float32
    g = np.asarray(inputs["ln_g"], f32)
    b = np.asarray(inputs["ln_b"], f32)
    Wp = np.asarray(inputs["Wp"], f32)
    bp = np.asarray(inputs["bp"], f32)
    Wr1 = np.asarray(inputs["Wr1"], f32)
    br1 = np.asarray(inputs["br1"], f32)
    Wr2 = np.asarray(inputs["Wr2"], f32)
    br2 = np.asarray(inputs["br2"], f32)
    Wf = np.asarray(inputs["Wf"], f32)
    bf = np.asarray(inputs["bf"], f32)
    We1h = np.asarray(inputs["We1h"], f32)
    We1f = np.asarray(inputs["We1f"], f32)
    be1 = np.asarray(inputs["be1"], f32)
    We2 = np.asarray(inputs["We2"], f32)
    be2 = np.asarray(inputs["be2"], f32)
    alpha = float(np.asarray(inputs["alpha"], f32))
    expert_idx = np.asarray(inputs["expert_idx"]).astype(np.int64)
    stage_idx = np.asarray(inputs["stage_idx"]).astype(np.int64)

    Wr1h, Wr1f = Wr1[:DM], Wr1[DM:]
    # stage gather folded: femb = feat[:, stage_idx] @ Wp + bp
    Mstage = np.zeros((NSF, NSF), f32)
    np.add.at(Mstage, stage_idx, Wp)
    w_r1h = g[:, None] * Wr1h
    w_r1f = Mstage @ Wr1f
    b_r1 = br1 + b @ Wr1h + bp @ Wr1f
    w_r2a = np.zeros((2 * DRH, 2 * E), f32)
    w_r2a[:DRH, :E] = Wr2
    w_r2a[DRH:, E:] = Wr2
    br2exp = np.tile(np.exp(br2)[None, :], (128, 1)).astype(f32)

    # expert gather + Wf@We1f collapsed; pairs (2i, 2i+1) packed in columns
    w_cmb = np.zeros((NSF, NPAIR * 128), f32)
    w_e1h = np.zeros((DM, NPAIR * 128), f32)
    b_e1 = np.zeros((128, NPAIR), f32)
    for e in range(E):
        Se = np.zeros((NSF, NF), f32)
        Se[expert_idx[e], np.arange(NF)] = 1.0
        cmb = (Se @ Wf[e]) @ We1f[e]                    # [64, 64]
        p, half = e // 2, (e % 2) * DH
        w_cmb[:, 128 * p + half:128 * p + half + DH] = cmb
        w_e1h[:, 128 * p + half:128 * p + half + DH] = g[:, None] * We1h[e]
        b_e1[half:half + DH, p] = be1[e] + bf[e] @ We1f[e] + b @ We1h[e]

    # We2 stacked [E*DH, DM] * alpha; chunk c rows = experts 2c,2c+1
    w_e2 = np.zeros((128, NPAIR * 128), f32)
    W2s = (alpha * We2).reshape(E * DH, DM)
    for c in range(NPAIR):
        w_e2[:, 128 * c:128 * (c + 1)] = W2s[128 * c:128 * (c + 1)]
    b_e2 = (alpha * be2).astype(f32)

    sel = np.zeros((E, NPAIR * 128), f32)
    for p in range(NPAIR):
        sel[2 * p, 128 * p:128 * p + 64] = 1.0
        sel[2 * p + 1, 128 * p + 64:128 * (p + 1)] = 1.0
    ident = np.eye(128, dtype=f32)
    bf = ml_dtypes.bfloat16
    return dict(w_r1h=w_r1h, w_r1f=w_r1f, b_r1=np.tile(b_r1, 2)[:, None],
                w_r2a=w_r2a, br2exp=br2exp,
                w_cmb=w_cmb.astype(bf), w_e1h=w_e1h.astype(bf), b_e1=b_e1,
                w_e2=w_e2.astype(bf), b_e2=b_e2.astype(bf),
                sel=sel.astype(bf), ident=ident)


def make_in_maps(inputs):
    wts = _fold_weights(inputs)
    hidden = np.ascontiguousarray(np.asarray(inputs["hidden"], np.float32))
    feat = np.ascontiguousarray(np.asarray(inputs["feat"], np.float32))
    seq = np.asarray(inputs["item_seq_len"]).astype(np.int64)

    in_maps = []
    for c in range(NCORES):
        bs = slice(BC * c, BC * (c + 1))
        hid_c = hidden[bs].reshape(NTOK, DM)
        feat_c = feat[bs].reshape(NTOK, NSF)
        # token (n, p): flat = n*128 + p; b_local = n // (T//128); t = flat % T
        n_idx = np.arange(NT)
        p_idx = np.arange(128)
        flat = n_idx[None, :] * 128 + p_idx[:, None]      # [128, nt]
        tpos = flat % T
        bloc = flat // T
        valid = (tpos < seq[bs][bloc]).astype(np.float32)
        m = dict(hidden=hid_c, feat=feat_c, valid=valid)
        m.update(wts)
        in_maps.append(m)
    return in_maps


LAST_RESULT = None


def kernel(**inputs):
    global LAST_RESULT
    nc = _built()
    in_maps = make_in_maps(inputs)
    res = run_bass_kernel_spmd(nc, in_maps, core_ids=list(range(NCORES)))
    LAST_RESULT = res
    nh = np.stack([res.results[c]["next_h"].reshape(BC, T, DM)
                   for c in range(NCORES)]).reshape(B, T, DM)
    ga = np.stack([res.results[c]["gates_o"].reshape(BC, T, E)
                   for c in range(NCORES)]).reshape(B, T, E)
    pr = np.stack([res.results[c]["probs_o"].reshape(BC, T, E)
                   for c in range(NCORES)]).reshape(B, T, E)
    return nh, ga, pr


# revision 17
# speedup vs baseline: 1.6873x; 1.6873x over previous
"""Trainium2 Bass kernel for nn_MoEStage (MoE routing stage).

Data-parallel over batch B=32 across 8 NeuronCores (4 batches/core).
All params replicated and pre-folded on host:
  - LayerNorm affine (ln_g, ln_b) folded into Wr1/We1h weights+biases
  - stage_idx/expert_idx gathers folded into weight matrices
  - Wf @ We1f collapsed into a single [64, 64] per-expert matrix
  - alpha folded into We2/be2

Device layout: token-major tiles [128 tokens, D] for LN/softmax/top-k,
feature-major [D, tokens] (via PE transposes) for all matmuls.
"""
import sys
import functools

sys.path.insert(0, "/opt/trn_rl_repo")

import numpy as np
import ml_dtypes

import concourse.bacc as bacc
import concourse.mybir as mybir
import concourse.tile as tile
from concourse.bass import broadcast_tensor_aps
from concourse.bass_utils import run_bass_kernel_spmd

F32 = mybir.dt.float32
F32R = mybir.dt.float32r
BF16 = mybir.dt.bfloat16

def r32(ap):
    return ap.bitcast(F32R)

AF = mybir.ActivationFunctionType
ALU = mybir.AluOpType
AX = mybir.AxisListType

B, T, DM, DFE, DH, DRH = 32, 2048, 128, 64, 64, 64
E, NF, NSF, TOPK = 8, 16, 64, 2
LN_EPS = 1e-5
NCORES = 8
BC = B // NCORES          # batches per core
NTOK = BC * T             # 8192 tokens per core
NT = NTOK // 128          # 64 token-tiles of 128
GSZ = 4                   # tiles per group
NG = NT // GSZ            # 16 groups of 512 tokens
GW = GSZ * 128            # 512 tokens per group
NPAIR = 4                 # expert pairs


def build_kernel(ntok=NTOK):
    nt = ntok // 128
    ng = nt // GSZ
    nc = bacc.Bacc("TRN2", target_bir_lowering=False, debug=False,
                   num_devices=NCORES)

    din = {}
    R32_INPUTS = set()
    BF_INPUTS = {"w_cmb", "w_e1h", "w_e2", "b_e2", "sel"}
    def inp(name, shape):
        dt = BF16 if name in BF_INPUTS else F32
        din[name] = nc.dram_tensor(name, list(shape), dt, kind="ExternalInput")
        return din[name]

    hidden = inp("hidden", [ntok, DM])
    feat = inp("feat", [ntok, NSF])
    valid = inp("valid", [128, nt])
    w_r1h = inp("w_r1h", [DM, DRH])          # lhsT K=128 -> M=64
    w_r1f = inp("w_r1f", [NSF, DRH])         # lhsT K=64 -> M=64
    b_r1 = inp("b_r1", [2 * DRH, 1])   # duplicated for col-packed pairs
    w_r2a = inp("w_r2a", [2 * DRH, 2 * E])   # block-diag [Wr2 0; 0 Wr2]
    br2exp = inp("br2exp", [128, E])
    w_cmb = inp("w_cmb", [NSF, NPAIR * 128])   # per pair: [64, 128]
    w_e1h = inp("w_e1h", [DM, NPAIR * 128])    # per pair: [128, 128]
    b_e1 = inp("b_e1", [128, NPAIR])
    w_e2 = inp("w_e2", [128, NPAIR * 128])     # chunk c: [K=128, M=128]
    b_e2 = inp("b_e2", [E, DM])
    sel = inp("sel", [E, NPAIR * 128])
    ident = inp("ident", [128, 128])

    next_h = nc.dram_tensor("next_h", [ntok, DM], F32, kind="ExternalOutput")
    gates_o = nc.dram_tensor("gates_o", [ntok, E], F32, kind="ExternalOutput")
    probs_o = nc.dram_tensor("probs_o", [ntok, E], F32, kind="ExternalOutput")

    with tile.TileContext(nc) as tc:
        with (
            tc.tile_pool(name="wp", bufs=1) as wp,
            tc.tile_pool(name="big", bufs=1) as big,
            tc.tile_pool(name="wk", bufs=3) as wk,
            tc.tile_pool(name="grp", bufs=2) as grp,
            tc.tile_pool(name="ps", bufs=2, space="PSUM") as ps,
            tc.tile_pool(name="ps1", bufs=1, space="PSUM") as ps1,
        ):
            # ---- load weights/constants ----
            wt = {}
            for name, h in din.items():
                if name in ("hidden", "feat"):
                    continue
                t = wp.tile(list(h.shape), h.dtype, tag=f"w_{name}")
                nc.sync.dma_start(t[:], h[:])
                wt[name] = t

            # ---- load activations (token-major, hidden padded for bn_stats) ----
            hid_tm = big.tile([128, nt, 132], F32)
            nc.sync.dma_start(hid_tm[:, :, 0:128],
                              hidden.ap().rearrange("(n p) d -> p n d", p=128))
            feat_tm = big.tile([128, nt, NSF], F32)
            nc.sync.dma_start(feat_tm[:],
                              feat.ap().rearrange("(n p) d -> p n d", p=128))

            # ---- LayerNorm stats via per-tile bn_stats ----
            bn = big.tile([128, nt, 8], F32)
            for n in range(nt):
                nc.vector.bn_stats(bn[:, n, 0:6], hid_tm[:, n, 0:128])
            me, ve = bn[:, :, 1], bn[:, :, 2]
            mo, vo = bn[:, :, 4], bn[:, :, 5]
            mu = big.tile([128, nt], F32)
            nc.vector.tensor_tensor(mu[:], me, mo, ALU.add)
            nc.vector.tensor_scalar_mul(mu[:], mu[:], 0.5)
            dmn = big.tile([128, nt], F32)
            nc.vector.tensor_tensor(dmn[:], me, mo, ALU.subtract)
            nc.vector.tensor_tensor(dmn[:], dmn[:], dmn[:], ALU.mult)
            var = big.tile([128, nt], F32)
            nc.vector.tensor_tensor(var[:], ve, vo, ALU.add)
            nc.vector.scalar_tensor_tensor(var[:], var[:], 1.0 / DM, dmn[:],
                                           ALU.mult, ALU.bypass)
            nc.vector.scalar_tensor_tensor(var[:], dmn[:], 0.25, var[:],
                                           ALU.mult, ALU.add)
            rstd = big.tile([128, nt], F32)
            epsb = wp.tile([128, 1], F32, tag="epsb")
            nc.vector.memset(epsb[:], LN_EPS)
            nc.scalar.activation(rstd[:], var[:], AF.Ln, bias=epsb[:])
            nc.scalar.activation(rstd[:], rstd[:], AF.Exp, scale=-0.5)

            identb = wp.tile([128, 128], BF16, tag="identb")
            nc.gpsimd.tensor_copy(identb[:], wt["ident"][:])

            h_fm = big.tile([128, ntok], F32)
            f_fm = big.tile([NSF, ntok], F32)
            logit_sb = big.tile([128, nt, E], F32)
            Eexp = big.tile([128, nt, E], F32)
            probs = big.tile([128, nt, E], F32)
            msk = big.tile([128, nt, E], F32)
            G0 = big.tile([128, nt, E], F32)
            gates = big.tile([128, nt, E], F32)
            gates_bf = big.tile([128, nt, E], BF16)
            g_fm = big.tile([E, ntok], BF16)
            S = big.tile([128, nt], F32)
            Sr = big.tile([128, nt], F32)
            m1 = big.tile([128, nt], F32)
            m2 = big.tile([128, nt], F32)
            dsum = big.tile([128, nt], F32)
            dr = big.tile([128, nt], F32)

            def bcol(t, sl):  # [128, cw] -> broadcastable [128, cw, E]
                return t[:, sl].rearrange("p (n o) -> p n o", o=1)

            NCH = 4                       # chunks
            CG = ng // NCH                # groups per chunk (4)
            CT = CG * GSZ                 # tiles per chunk (16)
            probs_od = probs_o.ap().rearrange("(n p) e -> p n e", p=128)
            gates_od = gates_o.ap().rearrange("(n p) e -> p n e", p=128)
            next_od = next_h.ap().rearrange("(n p) d -> p n d", p=128)

            for c in range(NCH):
                tsl = slice(CT * c, CT * (c + 1))          # tile slice
                # ---- normalize + transpose to feature-major ----
                for g in range(CG * c, CG * (c + 1)):
                    hps = ps.tile([128, GW], F32, tag="psA")
                    fps = ps.tile([NSF, GW], F32, tag="psB")
                    for s in range(GSZ):
                        n = GSZ * g + s
                        xn = wk.tile([128, 128], F32, tag="xn")
                        nc.vector.tensor_scalar(xn[:], hid_tm[:, n, 0:128],
                                                mu[:, n:n + 1], rstd[:, n:n + 1],
                                                ALU.subtract, ALU.mult)
                        nc.tensor.transpose(hps[:, 128 * s:128 * (s + 1)],
                                            xn[:], wt["ident"][:])
                        nc.tensor.transpose(fps[:, 128 * s:128 * (s + 1)],
                                            feat_tm[:, n, :], wt["ident"][:])
                    gsl = slice(GW * g, GW * (g + 1))
                    nc.scalar.copy(h_fm[:, gsl], hps[:])
                    nc.scalar.copy(f_fm[:, gsl], fps[:])

                # ---- router MLP (two groups packed per PE pass) ----
                for g2 in range(CG * c // 2, CG * (c + 1) // 2):
                    sl0 = slice(GW * 2 * g2, GW * (2 * g2 + 1))
                    sl1 = slice(GW * (2 * g2 + 1), GW * (2 * g2 + 2))
                    r1p = ps1.tile([128, 2, GW], F32, tag="psC")
                    nc.tensor.matmul(r1p[0:DRH, 0, :], wt["w_r1h"][:],
                                     h_fm[:, sl0], tile_position=(0, 0),
                                     start=True, stop=False)
                    nc.tensor.matmul(r1p[DRH:2 * DRH, 1, :], wt["w_r1h"][:],
                                     h_fm[:, sl1], tile_position=(0, 64),
                                     start=True, stop=False)
                    nc.tensor.matmul(r1p[0:DRH, 0, :], wt["w_r1f"][:],
                                     f_fm[:, sl0], tile_position=(0, 0),
                                     start=False, stop=True)
                    nc.tensor.matmul(r1p[DRH:2 * DRH, 1, :], wt["w_r1f"][:],
                                     f_fm[:, sl1], tile_position=(0, 64),
                                     start=False, stop=True)
                    r1sb = grp.tile([128, GW], F32, tag="r1sb")
                    nc.scalar.activation(r1sb[0:DRH, :], r1p[0:DRH, 0, :],
                                         AF.Relu, bias=wt["b_r1"][0:DRH, :])
                    nc.scalar.activation(r1sb[DRH:128, :], r1p[DRH:128, 1, :],
                                         AF.Relu, bias=wt["b_r1"][DRH:128, :])
                    logit_ps = ps1.tile([128, GSZ, 2, E], F32, tag="psD")
                    for s in range(GSZ):
                        nc.tensor.matmul(logit_ps[:, s, :, :],
                                         r1sb[:, 128 * s:128 * (s + 1)],
                                         wt["w_r2a"][:], start=True, stop=True)
                    dst = logit_sb[:, GSZ * 2 * g2:GSZ * (2 * g2 + 2), :]
                    nc.scalar.copy(dst.rearrange("p (h s) e -> p s h e", h=2),
                                   logit_ps[:])

                # ---- softmax (logits are O(1); no max-sub) + top-2 ----
                nc.scalar.activation(Eexp[:, tsl, :], logit_sb[:, tsl, :],
                                     AF.Exp)
                a, b_ = broadcast_tensor_aps(
                    Eexp[:, tsl, :],
                    wt["br2exp"][:].rearrange("p (n e) -> p n e", n=1))
                nc.vector.tensor_tensor(Eexp[:, tsl, :], a, b_, ALU.mult)
                nc.vector.reduce_sum(S[:, tsl], Eexp[:, tsl, :], axis=AX.X)
                nc.vector.reciprocal(Sr[:, tsl], S[:, tsl])
                a, b_ = broadcast_tensor_aps(Eexp[:, tsl, :], bcol(Sr, tsl))
                nc.vector.tensor_tensor(probs[:, tsl, :], a, b_, ALU.mult)
                nc.sync.dma_start(probs_od[:, tsl, :], probs[:, tsl, :])
                nc.vector.reduce_max(m1[:, tsl], Eexp[:, tsl, :], axis=AX.X)
                a, b_ = broadcast_tensor_aps(Eexp[:, tsl, :], bcol(m1, tsl))
                nc.vector.tensor_tensor(msk[:, tsl, :], a, b_, ALU.is_lt)
                nc.vector.tensor_tensor(G0[:, tsl, :], Eexp[:, tsl, :],
                                        msk[:, tsl, :], ALU.mult)
                nc.vector.reduce_max(m2[:, tsl], G0[:, tsl, :], axis=AX.X)
                a, b_ = broadcast_tensor_aps(Eexp[:, tsl, :], bcol(m2, tsl))
                nc.vector.tensor_tensor(msk[:, tsl, :], a, b_, ALU.is_ge)
                nc.vector.tensor_tensor(G0[:, tsl, :], Eexp[:, tsl, :],
                                        msk[:, tsl, :], ALU.mult)
                nc.vector.reduce_sum(dsum[:, tsl], G0[:, tsl, :], axis=AX.X)
                nc.vector.reciprocal(dr[:, tsl], dsum[:, tsl])
                nc.vector.tensor_tensor(dr[:, tsl], dr[:, tsl],
                                        wt["valid"][:, tsl], ALU.mult)
                a, b_ = broadcast_tensor_aps(G0[:, tsl, :], bcol(dr, tsl))
                nc.vector.tensor_tensor(gates[:, tsl, :], a, b_, ALU.mult)
                nc.sync.dma_start(gates_od[:, tsl, :], gates[:, tsl, :])

                # ---- gates to feature-major (bf16, combine-only) ----
                nc.vector.tensor_copy(gates_bf[:, tsl, :], gates[:, tsl, :])
                for g in range(CG * c, CG * (c + 1)):
                    gps = ps1.tile([E, GW], BF16, tag="psD")
                    for s in range(GSZ):
                        n = GSZ * g + s
                        nc.tensor.transpose(gps[:, 128 * s:128 * (s + 1)],
                                            gates_bf[:, n, :], identb[:])
                    nc.scalar.copy(g_fm[:, GW * g:GW * (g + 1)], gps[:])

                # ---- experts (dense, pairs of 2) + combine + residual ----
                for g in range(CG * c, CG * (c + 1)):
                    gsl = slice(GW * g, GW * (g + 1))
                    h_fr = grp.tile([128, GW], BF16, tag="h_fr")
                    nc.vector.tensor_copy(h_fr[:], h_fm[:, gsl])
                    f_fr = grp.tile([NSF, GW], BF16, tag="f_fr")
                    nc.vector.tensor_copy(f_fr[:], f_fm[:, gsl])
                    moe = ps1.tile([128, GW], F32, tag="psCm")
                    for p in range(NPAIR):
                        psl_ = slice(128 * p, 128 * (p + 1))
                        h1p = ps.tile([128, GW], F32, tag="psA")
                        nc.tensor.matmul(h1p[:], wt["w_e1h"][:, psl_],
                                         h_fr[:], start=True, stop=False)
                        nc.tensor.matmul(h1p[:], wt["w_cmb"][:, psl_],
                                         f_fr[:], start=False, stop=True)
                        h1 = grp.tile([128, GW], BF16, tag="h1")
                        nc.scalar.activation(h1[:], h1p[:], AF.Gelu,
                                             bias=wt["b_e1"][:, p:p + 1])
                        gb = ps.tile([128, GW], F32, tag="psB")
                        nc.tensor.matmul(gb[:], wt["sel"][:, psl_],
                                         g_fm[:, gsl], start=True, stop=True)
                        h1s = grp.tile([128, GW], BF16, tag="h1s")
                        nc.vector.tensor_tensor(h1s[:], h1[:], gb[:], ALU.mult)
                        nc.tensor.matmul(moe[:], wt["w_e2"][:, psl_], h1s[:],
                                         start=(p == 0), stop=False)
                    nc.tensor.matmul(moe[:], wt["b_e2"][:], g_fm[:, gsl],
                                     start=False, stop=True)
                    moe_sb = grp.tile([128, GW], BF16, tag="moe_sb")
                    nc.scalar.copy(moe_sb[:], moe[:])
                    moeT = grp.tile([128, GSZ, 128], BF16, tag="moeT")
                    nc.sync.dma_start_transpose(moeT[:], moe_sb[:])
                    nxt = grp.tile([128, GSZ, 128], F32, tag="nxt")
                    nc.vector.tensor_tensor(
                        nxt[:], hid_tm[:, GSZ * g:GSZ * (g + 1), 0:128],
                        moeT[:], ALU.add)
                    nc.sync.dma_start(next_od[:, GSZ * g:GSZ * (g + 1), :],
                                      nxt[:])

    nc.compile()
    return nc


@functools.cache
def _built():
    return build_kernel()


def _fold_weights(inputs):
    """Host-side folding of all parameters into device weight tensors."""
    f32 = np.float32
    g = np.asarray(inputs["ln_g"], f32)
    b = np.asarray(inputs["ln_b"], f32)
    Wp = np.asarray(inputs["Wp"], f32)
    bp = np.asarray(inputs["bp"], f32)
    Wr1 = np.asarray(inputs["Wr1"], f32)
    br1 = np.asarray(inputs["br1"], f32)
    Wr2 = np.asarray(inputs["Wr2"], f32)
    br2 = np.asarray(inputs["br2"], f32)
    Wf = np.asarray(inputs["Wf"], f32)
    bf = np.asarray(inputs["bf"], f32)
    We1h = np.asarray(inputs["We1h"], f32)
    We1f = np.asarray(inputs["We1f"], f32)
    be1 = np.asarray(inputs["be1"], f32)
    We2 = np.asarray(inputs["We2"], f32)
    be2 = np.asarray(inputs["be2"], f32)
    alpha = float(np.asarray(inputs["alpha"], f32))
    expert_idx = np.asarray(inputs["expert_idx"]).astype(np.int64)
    stage_idx = np.asarray(inputs["stage_idx"]).astype(np.int64)

    Wr1h, Wr1f = Wr1[:DM], Wr1[DM:]
    # stage gather folded: femb = feat[:, stage_idx] @ Wp + bp
    Mstage = np.zeros((NSF, NSF), f32)
    np.add.at(Mstage, stage_idx, Wp)
    w_r1h = g[:, None] * Wr1h
    w_r1f = Mstage @ Wr1f
    b_r1 = br1 + b @ Wr1h + bp @ Wr1f
    w_r2a = np.zeros((2 * DRH, 2 * E), f32)
    w_r2a[:DRH, :E] = Wr2
    w_r2a[DRH:, E:] = Wr2
    br2exp = np.tile(np.exp(br2)[None, :], (128, 1)).astype(f32)

    # expert gather + Wf@We1f collapsed; pairs (2i, 2i+1) packed in columns
    w_cmb = np.zeros((NSF, NPAIR * 128), f32)
    w_e1h = np.zeros((DM, NPAIR * 128), f32)
    b_e1 = np.zeros((128, NPAIR), f32)
    for e in range(E):
        Se = np.zeros((NSF, NF), f32)
        Se[expert_idx[e], np.arange(NF)] = 1.0
        cmb = (Se @ Wf[e]) @ We1f[e]                    # [64, 64]
        p, half = e // 2, (e % 2) * DH
        w_cmb[:, 128 * p + half:128 * p + half + DH] = cmb
        w_e1h[:, 128 * p + half:128 * p + half + DH] = g[:, None] * We1h[e]
        b_e1[half:half + DH, p] = be1[e] + bf[e] @ We1f[e] + b @ We1h[e]

    # We2 stacked [E*DH, DM] * alpha; chunk c rows = experts 2c,2c+1
    w_e2 = np.zeros((128, NPAIR * 128), f32)
    W2s = (alpha * We2).reshape(E * DH, DM)
    for c in range(NPAIR):
        w_e2[:, 128 * c:128 * (c + 1)] = W2s[128 * c:128 * (c + 1)]
    b_e2 = (alpha * be2).astype(f32)

    sel = np.zeros((E, NPAIR * 128), f32)
    for p in range(NPAIR):
        sel[2 * p, 128 * p:128 * p + 64] = 1.0
        sel[2 * p + 1, 128 * p + 64:128 * (p + 1)] = 1.0
    ident = np.eye(128, dtype=f32)
    bf = ml_dtypes.bfloat16
    return dict(w_r1h=w_r1h, w_r1f=w_r1f, b_r1=np.tile(b_r1, 2)[:, None],
                w_r2a=w_r2a, br2exp=br2exp,
                w_cmb=w_cmb.astype(bf), w_e1h=w_e1h.astype(bf), b_e1=b_e1,
                w_e2=w_e2.astype(bf), b_e2=b_e2.astype(bf),
                sel=sel.astype(bf), ident=ident)


def make_in_maps(inputs):
    wts = _fold_weights(inputs)
    hidden = np.ascontiguousarray(np.asarray(inputs["hidden"], np.float32))
    feat = np.ascontiguousarray(np.asarray(inputs["feat"], np.float32))
    seq = np.asarray(inputs["item_seq_len"]).astype(np.int64)

    in_maps = []
    for c in range(NCORES):
        bs = slice(BC * c, BC * (c + 1))
        hid_c = hidden[bs].reshape(NTOK, DM)
        feat_c = feat[bs].reshape(NTOK, NSF)
        # token (n, p): flat = n*128 + p; b_local = n // (T//128); t = flat % T
        n_idx = np.arange(NT)
        p_idx = np.arange(128)
        flat = n_idx[None, :] * 128 + p_idx[:, None]      # [128, nt]
        tpos = flat % T
        bloc = flat // T
        valid = (tpos < seq[bs][bloc]).astype(np.float32)
        m = dict(hidden=hid_c, feat=feat_c, valid=valid)
        m.update(wts)
        in_maps.append(m)
    return in_maps


LAST_RESULT = None


def kernel(**inputs):
    global LAST_RESULT
    nc = _built()
    in_maps = make_in_maps(inputs)
    res = run_bass_kernel_spmd(nc, in_maps, core_ids=list(range(NCORES)))
    LAST_RESULT = res
    nh = np.stack([res.results[c]["next_h"].reshape(BC, T, DM)
                   for c in range(NCORES)]).reshape(B, T, DM)
    ga = np.stack([res.results[c]["gates_o"].reshape(BC, T, E)
                   for c in range(NCORES)]).reshape(B, T, E)
    pr = np.stack([res.results[c]["probs_o"].reshape(BC, T, E)
                   for c in range(NCORES)]).reshape(B, T, E)
    return nh, ga, pr


# revision 21
# speedup vs baseline: 1.8163x; 1.0764x over previous
"""Trainium2 Bass kernel for nn_MoEStage (MoE routing stage).

Data-parallel over batch B=32 across 8 NeuronCores (4 batches/core).
All params replicated and pre-folded on host:
  - LayerNorm affine (ln_g, ln_b) folded into Wr1/We1h weights+biases
  - stage_idx/expert_idx gathers folded into weight matrices
  - Wf @ We1f collapsed into a single [64, 64] per-expert matrix
  - alpha folded into We2/be2

Device layout: token-major tiles [128 tokens, D] for LN/softmax/top-k,
feature-major [D, tokens] (via PE transposes) for all matmuls.
"""
import sys
import functools

sys.path.insert(0, "/opt/trn_rl_repo")

import numpy as np
import ml_dtypes

import concourse.bacc as bacc
import concourse.mybir as mybir
import concourse.tile as tile
from concourse.bass import broadcast_tensor_aps
from concourse.bass_utils import run_bass_kernel_spmd

F32 = mybir.dt.float32
F32R = mybir.dt.float32r
BF16 = mybir.dt.bfloat16

def r32(ap):
    return ap.bitcast(F32R)

AF = mybir.ActivationFunctionType
ALU = mybir.AluOpType
AX = mybir.AxisListType

B, T, DM, DFE, DH, DRH = 32, 2048, 128, 64, 64, 64
E, NF, NSF, TOPK = 8, 16, 64, 2
LN_EPS = 1e-5
NCORES = 8
BC = B // NCORES          # batches per core
NTOK = BC * T             # 8192 tokens per core
NT = NTOK // 128          # 64 token-tiles of 128
GSZ = 4                   # tiles per group
NG = NT // GSZ            # 16 groups of 512 tokens
GW = GSZ * 128            # 512 tokens per group
NPAIR = 4                 # expert pairs


def build_kernel(ntok=NTOK):
    nt = ntok // 128
    ng = nt // GSZ
    nc = bacc.Bacc("TRN2", target_bir_lowering=False, debug=False,
                   num_devices=NCORES)

    din = {}
    R32_INPUTS = set()
    BF_INPUTS = {"w_cmb", "w_e1h", "w_e2", "b_e2", "sel"}
    def inp(name, shape):
        dt = BF16 if name in BF_INPUTS else F32
        din[name] = nc.dram_tensor(name, list(shape), dt, kind="ExternalInput")
        return din[name]

    hidden = inp("hidden", [ntok, DM])
    feat = inp("feat", [ntok, NSF])
    valid = inp("valid", [128, nt])
    w_r1h = inp("w_r1h", [DM, DRH])          # lhsT K=128 -> M=64
    w_r1f = inp("w_r1f", [NSF, DRH])         # lhsT K=64 -> M=64
    b_r1 = inp("b_r1", [2 * DRH, 1])   # duplicated for col-packed pairs
    w_r2a = inp("w_r2a", [2 * DRH, 2 * E])   # block-diag [Wr2 0; 0 Wr2]
    br2exp = inp("br2exp", [128, E])
    w_cmb = inp("w_cmb", [NSF, NPAIR * 128])   # per pair: [64, 128]
    w_e1h = inp("w_e1h", [DM, NPAIR * 128])    # per pair: [128, 128]
    b_e1 = inp("b_e1", [128, NPAIR])
    w_e2 = inp("w_e2", [128, NPAIR * 128])     # chunk c: [K=128, M=128]
    b_e2 = inp("b_e2", [E, DM])
    sel = inp("sel", [E, NPAIR * 128])
    ident = inp("ident", [128, 128])

    next_h = nc.dram_tensor("next_h", [ntok, DM], F32, kind="ExternalOutput")
    gates_o = nc.dram_tensor("gates_o", [ntok, E], F32, kind="ExternalOutput")
    probs_o = nc.dram_tensor("probs_o", [ntok, E], F32, kind="ExternalOutput")

    with tile.TileContext(nc) as tc:
        with (
            tc.tile_pool(name="wp", bufs=1) as wp,
            tc.tile_pool(name="big", bufs=1) as big,
            tc.tile_pool(name="wk", bufs=3) as wk,
            tc.tile_pool(name="frp", bufs=1) as frp,
            tc.tile_pool(name="grp", bufs=2) as grp,
            tc.tile_pool(name="ps", bufs=2, space="PSUM") as ps,
            tc.tile_pool(name="ps1", bufs=1, space="PSUM") as ps1,
        ):
            # ---- load weights/constants ----
            wt = {}
            for name, h in din.items():
                if name in ("hidden", "feat"):
                    continue
                t = wp.tile(list(h.shape), h.dtype, tag=f"w_{name}")
                nc.sync.dma_start(t[:], h[:])
                wt[name] = t

            # ---- load activations (token-major, hidden padded for bn_stats) ----
            hid_tm = big.tile([128, nt, 132], F32)
            nc.sync.dma_start(hid_tm[:, :, 0:128],
                              hidden.ap().rearrange("(n p) d -> p n d", p=128))
            feat_tm = big.tile([128, nt, NSF], F32)
            nc.sync.dma_start(feat_tm[:],
                              feat.ap().rearrange("(n p) d -> p n d", p=128))

            # ---- LayerNorm stats via per-tile bn_stats ----
            bn = big.tile([128, nt, 8], F32)
            for n in range(nt):
                nc.vector.bn_stats(bn[:, n, 0:6], hid_tm[:, n, 0:128])
            me, ve = bn[:, :, 1], bn[:, :, 2]
            mo, vo = bn[:, :, 4], bn[:, :, 5]
            mu = big.tile([128, nt], F32)
            nc.vector.tensor_tensor(mu[:], me, mo, ALU.add)
            nc.vector.tensor_scalar_mul(mu[:], mu[:], 0.5)
            dmn = big.tile([128, nt], F32)
            nc.vector.tensor_tensor(dmn[:], me, mo, ALU.subtract)
            nc.vector.tensor_tensor(dmn[:], dmn[:], dmn[:], ALU.mult)
            var = big.tile([128, nt], F32)
            nc.vector.tensor_tensor(var[:], ve, vo, ALU.add)
            nc.vector.scalar_tensor_tensor(var[:], var[:], 1.0 / DM, dmn[:],
                                           ALU.mult, ALU.bypass)
            nc.vector.scalar_tensor_tensor(var[:], dmn[:], 0.25, var[:],
                                           ALU.mult, ALU.add)
            rstd = big.tile([128, nt], F32)
            epsb = wp.tile([128, 1], F32, tag="epsb")
            nc.vector.memset(epsb[:], LN_EPS)
            nc.scalar.activation(rstd[:], var[:], AF.Ln, bias=epsb[:])
            nc.scalar.activation(rstd[:], rstd[:], AF.Exp, scale=-0.5)

            identb = wp.tile([128, 128], BF16, tag="identb")
            nc.gpsimd.tensor_copy(identb[:], wt["ident"][:])

            h_fm = big.tile([128, ntok], F32)
            f_fm = big.tile([NSF, ntok], F32)
            logit_sb = big.tile([128, nt, E], F32)
            Eexp = big.tile([128, nt, E], F32)
            probs = big.tile([128, nt, E], F32)
            msk = big.tile([128, nt, E], F32)
            G0 = big.tile([128, nt, E], F32)
            gates = big.tile([128, nt, E], F32)
            gates_bf = big.tile([128, nt, E], BF16)
            g_fm = big.tile([E, ntok], BF16)
            S = big.tile([128, nt], F32)
            Sr = big.tile([128, nt], F32)
            m1 = big.tile([128, nt], F32)
            m2 = big.tile([128, nt], F32)
            dsum = big.tile([128, nt], F32)
            dr = big.tile([128, nt], F32)

            def bcol(t, sl):  # [128, cw] -> broadcastable [128, cw, E]
                return t[:, sl].rearrange("p (n o) -> p n o", o=1)

            NCH = 4                       # chunks
            CG = ng // NCH                # groups per chunk (4)
            CT = CG * GSZ                 # tiles per chunk (16)
            probs_od = probs_o.ap().rearrange("(n p) e -> p n e", p=128)
            gates_od = gates_o.ap().rearrange("(n p) e -> p n e", p=128)
            next_od = next_h.ap().rearrange("(n p) d -> p n d", p=128)

            def phase_A1(c):
                # normalize + transpose to feature-major (+ bf16 expert copies)
                for g in range(CG * c, CG * (c + 1)):
                    hps = ps.tile([128, GW], F32, tag="psA")
                    fps = ps.tile([NSF, GW], F32, tag="psB")
                    for s in range(GSZ):
                        n = GSZ * g + s
                        xn = wk.tile([128, 128], F32, tag="xn")
                        nc.vector.tensor_scalar(xn[:], hid_tm[:, n, 0:128],
                                                mu[:, n:n + 1], rstd[:, n:n + 1],
                                                ALU.subtract, ALU.mult)
                        nc.tensor.transpose(hps[:, 128 * s:128 * (s + 1)],
                                            xn[:], wt["ident"][:])
                        nc.tensor.transpose(fps[:, 128 * s:128 * (s + 1)],
                                            feat_tm[:, n, :], wt["ident"][:])
                    gsl = slice(GW * g, GW * (g + 1))
                    nc.scalar.copy(h_fm[:, gsl], hps[:])
                    nc.scalar.copy(f_fm[:, gsl], fps[:])
                    lsl = slice(GW * g - CW * c, GW * (g + 1) - CW * c)
                    nc.vector.tensor_copy(h_fr[c][:, lsl], h_fm[:, gsl])
                    nc.vector.tensor_copy(f_fr[c][:, lsl], f_fm[:, gsl])

            def phase_router(c):
                for g2 in range(CG * c // 2, CG * (c + 1) // 2):
                    sl0 = slice(GW * 2 * g2, GW * (2 * g2 + 1))
                    sl1 = slice(GW * (2 * g2 + 1), GW * (2 * g2 + 2))
                    r1p = ps1.tile([128, 2, GW], F32, tag="psC")
                    nc.tensor.matmul(r1p[0:DRH, 0, :], wt["w_r1h"][:],
                                     h_fm[:, sl0], tile_position=(0, 0),
                                     start=True, stop=False)
                    nc.tensor.matmul(r1p[DRH:2 * DRH, 1, :], wt["w_r1h"][:],
                                     h_fm[:, sl1], tile_position=(0, 64),
                                     start=True, stop=False)
                    nc.tensor.matmul(r1p[0:DRH, 0, :], wt["w_r1f"][:],
                                     f_fm[:, sl0], tile_position=(0, 0),
                                     start=False, stop=True)
                    nc.tensor.matmul(r1p[DRH:2 * DRH, 1, :], wt["w_r1f"][:],
                                     f_fm[:, sl1], tile_position=(0, 64),
                                     start=False, stop=True)
                    r1sb = grp.tile([128, GW], F32, tag="r1sb")
                    nc.scalar.activation(r1sb[0:DRH, :], r1p[0:DRH, 0, :],
                                         AF.Relu, bias=wt["b_r1"][0:DRH, :])
                    nc.scalar.activation(r1sb[DRH:128, :], r1p[DRH:128, 1, :],
                                         AF.Relu, bias=wt["b_r1"][DRH:128, :])
                    logit_ps = ps1.tile([128, GSZ, 2, E], F32, tag="psD")
                    for s in range(GSZ):
                        nc.tensor.matmul(logit_ps[:, s, :, :],
                                         r1sb[:, 128 * s:128 * (s + 1)],
                                         wt["w_r2a"][:], start=True, stop=True)
                    dst = logit_sb[:, GSZ * 2 * g2:GSZ * (2 * g2 + 2), :]
                    nc.scalar.copy(dst.rearrange("p (h s) e -> p s h e", h=2),
                                   logit_ps[:])

            def phase_softmax(c):
                tsl = slice(CT * c, CT * (c + 1))
                nc.scalar.activation(Eexp[:, tsl, :], logit_sb[:, tsl, :],
                                     AF.Exp)
                a, b_ = broadcast_tensor_aps(
                    Eexp[:, tsl, :],
                    wt["br2exp"][:].rearrange("p (n e) -> p n e", n=1))
                nc.vector.tensor_tensor(Eexp[:, tsl, :], a, b_, ALU.mult)
                nc.vector.reduce_sum(S[:, tsl], Eexp[:, tsl, :], axis=AX.X)
                nc.vector.reciprocal(Sr[:, tsl], S[:, tsl])
                a, b_ = broadcast_tensor_aps(Eexp[:, tsl, :], bcol(Sr, tsl))
                nc.vector.tensor_tensor(probs[:, tsl, :], a, b_, ALU.mult)
                nc.sync.dma_start(probs_od[:, tsl, :], probs[:, tsl, :])
                nc.vector.reduce_max(m1[:, tsl], Eexp[:, tsl, :], axis=AX.X)
                a, b_ = broadcast_tensor_aps(Eexp[:, tsl, :], bcol(m1, tsl))
                nc.vector.tensor_tensor(msk[:, tsl, :], a, b_, ALU.is_lt)
                nc.vector.tensor_tensor(G0[:, tsl, :], Eexp[:, tsl, :],
                                        msk[:, tsl, :], ALU.mult)
                nc.vector.reduce_max(m2[:, tsl], G0[:, tsl, :], axis=AX.X)
                a, b_ = broadcast_tensor_aps(Eexp[:, tsl, :], bcol(m2, tsl))
                nc.vector.tensor_tensor(msk[:, tsl, :], a, b_, ALU.is_ge)
                nc.vector.tensor_tensor(G0[:, tsl, :], Eexp[:, tsl, :],
                                        msk[:, tsl, :], ALU.mult)
                nc.vector.reduce_sum(dsum[:, tsl], G0[:, tsl, :], axis=AX.X)
                nc.vector.reciprocal(dr[:, tsl], dsum[:, tsl])
                nc.vector.tensor_tensor(dr[:, tsl], dr[:, tsl],
                                        wt["valid"][:, tsl], ALU.mult)
                a, b_ = broadcast_tensor_aps(G0[:, tsl, :], bcol(dr, tsl))
                nc.vector.tensor_tensor(gates[:, tsl, :], a, b_, ALU.mult)
                nc.sync.dma_start(gates_od[:, tsl, :], gates[:, tsl, :])
                nc.vector.tensor_copy(gates_bf[:, tsl, :], gates[:, tsl, :])

            def phase_gfm(c):
                for g in range(CG * c, CG * (c + 1)):
                    gps = ps1.tile([E, GW], BF16, tag="psD")
                    for s in range(GSZ):
                        n = GSZ * g + s
                        nc.tensor.transpose(gps[:, 128 * s:128 * (s + 1)],
                                            gates_bf[:, n, :], identb[:])
                    nc.scalar.copy(g_fm[:, GW * g:GW * (g + 1)], gps[:])

            def phase_experts(c):
                for g in range(CG * c, CG * (c + 1)):
                    gsl = slice(GW * g, GW * (g + 1))
                    moe = ps1.tile([128, GW], F32, tag="psCm")
                    for p in range(NPAIR):
                        psl_ = slice(128 * p, 128 * (p + 1))
                        h1p = ps.tile([128, GW], F32, tag="psA")
                        lsl = slice(GW * g - CW * c, GW * (g + 1) - CW * c)
                        nc.tensor.matmul(h1p[:], wt["w_e1h"][:, psl_],
                                         h_fr[c][:, lsl], start=True, stop=False)
                        nc.tensor.matmul(h1p[:], wt["w_cmb"][:, psl_],
                                         f_fr[c][:, lsl], start=False, stop=True)
                        h1 = grp.tile([128, GW], BF16, tag="h1")
                        nc.scalar.activation(h1[:], h1p[:], AF.Gelu,
                                             bias=wt["b_e1"][:, p:p + 1])
                        gb = ps.tile([128, GW], F32, tag="psB")
                        nc.tensor.matmul(gb[:], wt["sel"][:, psl_],
                                         g_fm[:, gsl], start=True, stop=True)
                        h1s = grp.tile([128, GW], BF16, tag="h1s")
                        nc.vector.tensor_tensor(h1s[:], h1[:], gb[:], ALU.mult)
                        nc.tensor.matmul(moe[:], wt["w_e2"][:, psl_], h1s[:],
                                         start=(p == 0), stop=False)
                    nc.tensor.matmul(moe[:], wt["b_e2"][:], g_fm[:, gsl],
                                     start=False, stop=True)
                    moe_sb = grp.tile([128, GW], BF16, tag="moe_sb")
                    nc.scalar.copy(moe_sb[:], moe[:])
                    moeT = grp.tile([128, GSZ, 128], BF16, tag="moeT")
                    nc.sync.dma_start_transpose(moeT[:], moe_sb[:])
                    nxt = grp.tile([128, GSZ, 128], F32, tag="nxt")
                    nc.vector.tensor_tensor(
                        nxt[:], hid_tm[:, GSZ * g:GSZ * (g + 1), 0:128],
                        moeT[:], ALU.add)
                    nc.sync.dma_start(next_od[:, GSZ * g:GSZ * (g + 1), :],
                                      nxt[:])

            # bf16 expert-input copies, double-buffered per chunk (parity)
            CW = CT * 128                 # tokens per chunk
            h_fr = {}
            f_fr = {}
            for c in range(NCH):
                h_fr[c] = frp.tile([128, CW], BF16, tag=f"h_fr{c % 2}", name=f"h_fr{c}")
                f_fr[c] = frp.tile([NSF, CW], BF16, tag=f"f_fr{c % 2}", name=f"f_fr{c}")

            # software pipeline: next chunk's transposes/router are emitted
            # before this chunk's gates/experts so PE never starves
            phase_A1(0)
            phase_router(0)
            phase_softmax(0)
            for c in range(NCH):
                if c + 1 < NCH:
                    phase_A1(c + 1)
                    phase_router(c + 1)
                phase_gfm(c)
                if c + 1 < NCH:
                    phase_softmax(c + 1)
                phase_experts(c)

    nc.compile()
    return nc


@functools.cache
def _built():
    return build_kernel()


def _fold_weights(inputs):
    """Host-side folding of all parameters into device weight tensors."""
    f32 = np.float32
    g = np.asarray(inputs["ln_g"], f32)
    b = np.asarray(inputs["ln_b"], f32)
    Wp = np.asarray(inputs["Wp"], f32)
    bp = np.asarray(inputs["bp"], f32)
    Wr1 = np.asarray(inputs["Wr1"], f32)
    br1 = np.asarray(inputs["br1"], f32)
    Wr2 = np.asarray(inputs["Wr2"], f32)
    br2 = np.asarray(inputs["br2"], f32)
    Wf = np.asarray(inputs["Wf"], f32)
    bf = np.asarray(inputs["bf"], f32)
    We1h = np.asarray(inputs["We1h"], f32)
    We1f = np.asarray(inputs["We1f"], f32)
    be1 = np.asarray(inputs["be1"], f32)
    We2 = np.asarray(inputs["We2"], f32)
    be2 = np.asarray(inputs["be2"], f32)
    alpha = float(np.asarray(inputs["alpha"], f32))
    expert_idx = np.asarray(inputs["expert_idx"]).astype(np.int64)
    stage_idx = np.asarray(inputs["stage_idx"]).astype(np.int64)

    Wr1h, Wr1f = Wr1[:DM], Wr1[DM:]
    # stage gather folded: femb = feat[:, stage_idx] @ Wp + bp
    Mstage = np.zeros((NSF, NSF), f32)
    np.add.at(Mstage, stage_idx, Wp)
    w_r1h = g[:, None] * Wr1h
    w_r1f = Mstage @ Wr1f
    b_r1 = br1 + b @ Wr1h + bp @ Wr1f
    w_r2a = np.zeros((2 * DRH, 2 * E), f32)
    w_r2a[:DRH, :E] = Wr2
    w_r2a[DRH:, E:] = Wr2
    br2exp = np.tile(np.exp(br2)[None, :], (128, 1)).astype(f32)

    # expert gather + Wf@We1f collapsed; pairs (2i, 2i+1) packed in columns
    w_cmb = np.zeros((NSF, NPAIR * 128), f32)
    w_e1h = np.zeros((DM, NPAIR * 128), f32)
    b_e1 = np.zeros((128, NPAIR), f32)
    for e in range(E):
        Se = np.zeros((NSF, NF), f32)
        Se[expert_idx[e], np.arange(NF)] = 1.0
        cmb = (Se @ Wf[e]) @ We1f[e]                    # [64, 64]
        p, half = e // 2, (e % 2) * DH
        w_cmb[:, 128 * p + half:128 * p + half + DH] = cmb
        w_e1h[:, 128 * p + half:128 * p + half + DH] = g[:, None] * We1h[e]
        b_e1[half:half + DH, p] = be1[e] + bf[e] @ We1f[e] + b @ We1h[e]

    # We2 stacked [E*DH, DM] * alpha; chunk c rows = experts 2c,2c+1
    w_e2 = np.zeros((128, NPAIR * 128), f32)
    W2s = (alpha * We2).reshape(E * DH, DM)
    for c in range(NPAIR):
        w_e2[:, 128 * c:128 * (c + 1)] = W2s[128 * c:128 * (c + 1)]
    b_e2 = (alpha * be2).astype(f32)

    sel = np.zeros((E, NPAIR * 128), f32)
    for p in range(NPAIR):
        sel[2 * p, 128 * p:128 * p + 64] = 1.0
        sel[2 * p + 1, 128 * p + 64:128 * (p + 1)] = 1.0
    ident = np.eye(128, dtype=f32)
    bf = ml_dtypes.bfloat16
    return dict(w_r1h=w_r1h, w_r1f=w_r1f, b_r1=np.tile(b_r1, 2)[:, None],
                w_r2a=w_r2a, br2exp=br2exp,
                w_cmb=w_cmb.astype(bf), w_e1h=w_e1h.astype(bf), b_e1=b_e1,
                w_e2=w_e2.astype(bf), b_e2=b_e2.astype(bf),
                sel=sel.astype(bf), ident=ident)


def make_in_maps(inputs):
    wts = _fold_weights(inputs)
    hidden = np.ascontiguousarray(np.asarray(inputs["hidden"], np.float32))
    feat = np.ascontiguousarray(np.asarray(inputs["feat"], np.float32))
    seq = np.asarray(inputs["item_seq_len"]).astype(np.int64)

    in_maps = []
    for c in range(NCORES):
        bs = slice(BC * c, BC * (c + 1))
        hid_c = hidden[bs].reshape(NTOK, DM)
        feat_c = feat[bs].reshape(NTOK, NSF)
        # token (n, p): flat = n*128 + p; b_local = n // (T//128); t = flat % T
        n_idx = np.arange(NT)
        p_idx = np.arange(128)
        flat = n_idx[None, :] * 128 + p_idx[:, None]      # [128, nt]
        tpos = flat % T
        bloc = flat // T
        valid = (tpos < seq[bs][bloc]).astype(np.float32)
        m = dict(hidden=hid_c, feat=feat_c, valid=valid)
        m.update(wts)
        in_maps.append(m)
    return in_maps


LAST_RESULT = None


def kernel(**inputs):
    global LAST_RESULT
    nc = _built()
    in_maps = make_in_maps(inputs)
    res = run_bass_kernel_spmd(nc, in_maps, core_ids=list(range(NCORES)))
    LAST_RESULT = res
    nh = np.stack([res.results[c]["next_h"].reshape(BC, T, DM)
                   for c in range(NCORES)]).reshape(B, T, DM)
    ga = np.stack([res.results[c]["gates_o"].reshape(BC, T, E)
                   for c in range(NCORES)]).reshape(B, T, E)
    pr = np.stack([res.results[c]["probs_o"].reshape(BC, T, E)
                   for c in range(NCORES)]).reshape(B, T, E)
    return nh, ga, pr


# revision 22
# speedup vs baseline: 1.8931x; 1.0423x over previous
"""Trainium2 Bass kernel for nn_MoEStage (MoE routing stage).

Data-parallel over batch B=32 across 8 NeuronCores (4 batches/core).
All params replicated and pre-folded on host:
  - LayerNorm affine (ln_g, ln_b) folded into Wr1/We1h weights+biases
  - stage_idx/expert_idx gathers folded into weight matrices
  - Wf @ We1f collapsed into a single [64, 64] per-expert matrix
  - alpha folded into We2/be2

Device layout: token-major tiles [128 tokens, D] for LN/softmax/top-k,
feature-major [D, tokens] (via PE transposes) for all matmuls.
"""
import sys
import functools

sys.path.insert(0, "/opt/trn_rl_repo")

import numpy as np
import ml_dtypes

import concourse.bacc as bacc
import concourse.mybir as mybir
import concourse.tile as tile
from concourse.bass import broadcast_tensor_aps
from concourse.bass_utils import run_bass_kernel_spmd

F32 = mybir.dt.float32
F32R = mybir.dt.float32r
BF16 = mybir.dt.bfloat16

def r32(ap):
    return ap.bitcast(F32R)

AF = mybir.ActivationFunctionType
ALU = mybir.AluOpType
AX = mybir.AxisListType

B, T, DM, DFE, DH, DRH = 32, 2048, 128, 64, 64, 64
E, NF, NSF, TOPK = 8, 16, 64, 2
LN_EPS = 1e-5
NCORES = 8
BC = B // NCORES          # batches per core
NTOK = BC * T             # 8192 tokens per core
NT = NTOK // 128          # 64 token-tiles of 128
GSZ = 4                   # tiles per group
NG = NT // GSZ            # 16 groups of 512 tokens
GW = GSZ * 128            # 512 tokens per group
NPAIR = 4                 # expert pairs


def build_kernel(ntok=NTOK):
    nt = ntok // 128
    ng = nt // GSZ
    nc = bacc.Bacc("TRN2", target_bir_lowering=False, debug=False,
                   num_devices=NCORES)

    din = {}
    R32_INPUTS = set()
    BF_INPUTS = {"w_cmb", "w_e1h", "w_e2", "b_e2", "sel"}
    def inp(name, shape):
        dt = BF16 if name in BF_INPUTS else F32
        din[name] = nc.dram_tensor(name, list(shape), dt, kind="ExternalInput")
        return din[name]

    hidden = inp("hidden", [ntok, DM])
    feat = inp("feat", [ntok, NSF])
    valid = inp("valid", [128, nt])
    w_r1h = inp("w_r1h", [DM, DRH])          # lhsT K=128 -> M=64
    w_r1f = inp("w_r1f", [NSF, DRH])         # lhsT K=64 -> M=64
    b_r1 = inp("b_r1", [2 * DRH, 1])   # duplicated for col-packed pairs
    w_r2a = inp("w_r2a", [2 * DRH, 2 * E])   # block-diag [Wr2 0; 0 Wr2]
    br2exp = inp("br2exp", [128, E])
    w_cmb = inp("w_cmb", [NSF, NPAIR * 128])   # per pair: [64, 128]
    w_e1h = inp("w_e1h", [DM, NPAIR * 128])    # per pair: [128, 128]
    b_e1 = inp("b_e1", [128, NPAIR])
    w_e2 = inp("w_e2", [128, NPAIR * 128])     # chunk c: [K=128, M=128]
    b_e2 = inp("b_e2", [E, DM])
    sel = inp("sel", [E, NPAIR * 128])
    ident = inp("ident", [128, 128])

    next_h = nc.dram_tensor("next_h", [ntok, DM], F32, kind="ExternalOutput")
    gates_o = nc.dram_tensor("gates_o", [ntok, E], F32, kind="ExternalOutput")
    probs_o = nc.dram_tensor("probs_o", [ntok, E], F32, kind="ExternalOutput")

    with tile.TileContext(nc) as tc:
        with (
            tc.tile_pool(name="wp", bufs=1) as wp,
            tc.tile_pool(name="big", bufs=1) as big,
            tc.tile_pool(name="wk", bufs=3) as wk,
            tc.tile_pool(name="frp", bufs=1) as frp,
            tc.tile_pool(name="grp", bufs=2) as grp,
            tc.tile_pool(name="ps", bufs=2, space="PSUM") as ps,
            tc.tile_pool(name="ps1", bufs=1, space="PSUM") as ps1,
        ):
            # ---- load weights/constants ----
            wt = {}
            for name, h in din.items():
                if name in ("hidden", "feat"):
                    continue
                t = wp.tile(list(h.shape), h.dtype, tag=f"w_{name}")
                nc.sync.dma_start(t[:], h[:])
                wt[name] = t

            # ---- load activations (token-major, hidden padded for bn_stats) ----
            hid_tm = big.tile([128, nt, 132], F32)
            nc.sync.dma_start(hid_tm[:, :, 0:128],
                              hidden.ap().rearrange("(n p) d -> p n d", p=128))
            feat_tm = big.tile([128, nt, NSF], F32)
            nc.sync.dma_start(feat_tm[:],
                              feat.ap().rearrange("(n p) d -> p n d", p=128))

            # ---- LayerNorm stats via per-tile bn_stats ----
            bn = big.tile([128, nt, 8], F32)
            for n in range(nt):
                nc.vector.bn_stats(bn[:, n, 0:6], hid_tm[:, n, 0:128])
            me, ve = bn[:, :, 1], bn[:, :, 2]
            mo, vo = bn[:, :, 4], bn[:, :, 5]
            mu = big.tile([128, nt], F32)
            nc.vector.tensor_tensor(mu[:], me, mo, ALU.add)
            nc.vector.tensor_scalar_mul(mu[:], mu[:], 0.5)
            dmn = big.tile([128, nt], F32)
            nc.vector.tensor_tensor(dmn[:], me, mo, ALU.subtract)
            nc.vector.tensor_tensor(dmn[:], dmn[:], dmn[:], ALU.mult)
            var = big.tile([128, nt], F32)
            nc.vector.tensor_tensor(var[:], ve, vo, ALU.add)
            nc.vector.scalar_tensor_tensor(var[:], var[:], 1.0 / DM, dmn[:],
                                           ALU.mult, ALU.bypass)
            nc.vector.scalar_tensor_tensor(var[:], dmn[:], 0.25, var[:],
                                           ALU.mult, ALU.add)
            rstd = big.tile([128, nt], F32)
            epsb = wp.tile([128, 1], F32, tag="epsb")
            nc.vector.memset(epsb[:], LN_EPS)
            nc.scalar.activation(rstd[:], var[:], AF.Ln, bias=epsb[:])
            nc.scalar.activation(rstd[:], rstd[:], AF.Exp, scale=-0.5)

            identb = wp.tile([128, 128], BF16, tag="identb")
            nc.gpsimd.tensor_copy(identb[:], wt["ident"][:])

            h_fm = big.tile([128, ntok], F32)
            f_fm = big.tile([NSF, ntok], F32)
            logit_sb = big.tile([128, nt, E], F32)
            Eexp = big.tile([128, nt, E], F32)
            probs = big.tile([128, nt, E], F32)
            msk = big.tile([128, nt, E], F32)
            G0 = big.tile([128, nt, E], F32)
            gates = big.tile([128, nt, E], F32)
            gates_bf = big.tile([128, nt, E], BF16)
            g_fm = big.tile([E, ntok], BF16)
            S = big.tile([128, nt], F32)
            Sr = big.tile([128, nt], F32)
            m1 = big.tile([128, nt], F32)
            m2 = big.tile([128, nt], F32)
            dsum = big.tile([128, nt], F32)
            dr = big.tile([128, nt], F32)

            def bcol(t, sl):  # [128, cw] -> broadcastable [128, cw, E]
                return t[:, sl].rearrange("p (n o) -> p n o", o=1)

            NCH = 4                       # chunks
            CG = ng // NCH                # groups per chunk (4)
            CT = CG * GSZ                 # tiles per chunk (16)
            probs_od = probs_o.ap().rearrange("(n p) e -> p n e", p=128)
            gates_od = gates_o.ap().rearrange("(n p) e -> p n e", p=128)
            next_od = next_h.ap().rearrange("(n p) d -> p n d", p=128)

            def phase_A1(c):
                # normalize + transpose to feature-major (+ bf16 expert copies)
                for g in range(CG * c, CG * (c + 1)):
                    hps = ps.tile([128, GW], F32, tag="psA")
                    fps = ps.tile([NSF, GW], F32, tag="psB")
                    for s in range(GSZ):
                        n = GSZ * g + s
                        xn = wk.tile([128, 128], F32, tag="xn", bufs=4)
                        nc.vector.tensor_scalar(xn[:], hid_tm[:, n, 0:128],
                                                mu[:, n:n + 1], rstd[:, n:n + 1],
                                                ALU.subtract, ALU.mult)
                        nc.tensor.transpose(hps[:, 128 * s:128 * (s + 1)],
                                            xn[:], wt["ident"][:])
                        nc.tensor.transpose(fps[:, 128 * s:128 * (s + 1)],
                                            feat_tm[:, n, :], wt["ident"][:])
                    gsl = slice(GW * g, GW * (g + 1))
                    nc.scalar.copy(h_fm[:, gsl], hps[:])
                    nc.scalar.copy(f_fm[:, gsl], fps[:])
                    lsl = slice(GW * g - CW * c, GW * (g + 1) - CW * c)
                    nc.vector.tensor_copy(h_fr[c][:, lsl], h_fm[:, gsl])
                    nc.vector.tensor_copy(f_fr[c][:, lsl], f_fm[:, gsl])

            def phase_router(c):
                for g2 in range(CG * c // 2, CG * (c + 1) // 2):
                    sl0 = slice(GW * 2 * g2, GW * (2 * g2 + 1))
                    sl1 = slice(GW * (2 * g2 + 1), GW * (2 * g2 + 2))
                    r1p = ps1.tile([128, 2, GW], F32, tag="psC")
                    nc.tensor.matmul(r1p[0:DRH, 0, :], wt["w_r1h"][:],
                                     h_fm[:, sl0], tile_position=(0, 0),
                                     start=True, stop=False)
                    nc.tensor.matmul(r1p[DRH:2 * DRH, 1, :], wt["w_r1h"][:],
                                     h_fm[:, sl1], tile_position=(0, 64),
                                     start=True, stop=False)
                    nc.tensor.matmul(r1p[0:DRH, 0, :], wt["w_r1f"][:],
                                     f_fm[:, sl0], tile_position=(0, 0),
                                     start=False, stop=True)
                    nc.tensor.matmul(r1p[DRH:2 * DRH, 1, :], wt["w_r1f"][:],
                                     f_fm[:, sl1], tile_position=(0, 64),
                                     start=False, stop=True)
                    r1sb = grp.tile([128, GW], F32, tag="r1sb")
                    nc.scalar.activation(r1sb[0:DRH, :], r1p[0:DRH, 0, :],
                                         AF.Relu, bias=wt["b_r1"][0:DRH, :])
                    nc.scalar.activation(r1sb[DRH:128, :], r1p[DRH:128, 1, :],
                                         AF.Relu, bias=wt["b_r1"][DRH:128, :])
                    logit_ps = ps1.tile([128, GSZ, 2, E], F32, tag="psD")
                    for s in range(GSZ):
                        nc.tensor.matmul(logit_ps[:, s, :, :],
                                         r1sb[:, 128 * s:128 * (s + 1)],
                                         wt["w_r2a"][:], start=True, stop=True)
                    dst = logit_sb[:, GSZ * 2 * g2:GSZ * (2 * g2 + 2), :]
                    nc.scalar.copy(dst.rearrange("p (h s) e -> p s h e", h=2),
                                   logit_ps[:])

            def phase_softmax(c):
                tsl = slice(CT * c, CT * (c + 1))
                nc.scalar.activation(Eexp[:, tsl, :], logit_sb[:, tsl, :],
                                     AF.Exp)
                a, b_ = broadcast_tensor_aps(
                    Eexp[:, tsl, :],
                    wt["br2exp"][:].rearrange("p (n e) -> p n e", n=1))
                nc.vector.tensor_tensor(Eexp[:, tsl, :], a, b_, ALU.mult)
                nc.vector.reduce_sum(S[:, tsl], Eexp[:, tsl, :], axis=AX.X)
                nc.vector.reciprocal(Sr[:, tsl], S[:, tsl])
                a, b_ = broadcast_tensor_aps(Eexp[:, tsl, :], bcol(Sr, tsl))
                nc.vector.tensor_tensor(probs[:, tsl, :], a, b_, ALU.mult)
                nc.sync.dma_start(probs_od[:, tsl, :], probs[:, tsl, :])
                nc.vector.reduce_max(m1[:, tsl], Eexp[:, tsl, :], axis=AX.X)
                a, b_ = broadcast_tensor_aps(Eexp[:, tsl, :], bcol(m1, tsl))
                nc.vector.tensor_tensor(msk[:, tsl, :], a, b_, ALU.is_lt)
                nc.vector.tensor_tensor(G0[:, tsl, :], Eexp[:, tsl, :],
                                        msk[:, tsl, :], ALU.mult)
                nc.vector.reduce_max(m2[:, tsl], G0[:, tsl, :], axis=AX.X)
                a, b_ = broadcast_tensor_aps(Eexp[:, tsl, :], bcol(m2, tsl))
                nc.vector.tensor_tensor(msk[:, tsl, :], a, b_, ALU.is_ge)
                nc.vector.tensor_tensor(G0[:, tsl, :], Eexp[:, tsl, :],
                                        msk[:, tsl, :], ALU.mult)
                nc.vector.reduce_sum(dsum[:, tsl], G0[:, tsl, :], axis=AX.X)
                nc.vector.reciprocal(dr[:, tsl], dsum[:, tsl])
                nc.vector.tensor_tensor(dr[:, tsl], dr[:, tsl],
                                        wt["valid"][:, tsl], ALU.mult)
                a, b_ = broadcast_tensor_aps(G0[:, tsl, :], bcol(dr, tsl))
                nc.vector.tensor_tensor(gates[:, tsl, :], a, b_, ALU.mult)
                nc.sync.dma_start(gates_od[:, tsl, :], gates[:, tsl, :])
                nc.vector.tensor_copy(gates_bf[:, tsl, :], gates[:, tsl, :])

            def phase_gfm(c):
                for g in range(CG * c, CG * (c + 1)):
                    gps = ps1.tile([E, GW], BF16, tag="psD")
                    for s in range(GSZ):
                        n = GSZ * g + s
                        nc.tensor.transpose(gps[:, 128 * s:128 * (s + 1)],
                                            gates_bf[:, n, :], identb[:])
                    nc.scalar.copy(g_fm[:, GW * g:GW * (g + 1)], gps[:])

            def phase_experts(c):
                for g in range(CG * c, CG * (c + 1)):
                    gsl = slice(GW * g, GW * (g + 1))
                    moe = ps1.tile([128, GW], F32, tag="psCm")
                    for p in range(NPAIR):
                        psl_ = slice(128 * p, 128 * (p + 1))
                        h1p = ps.tile([128, GW], F32, tag="psA")
                        lsl = slice(GW * g - CW * c, GW * (g + 1) - CW * c)
                        nc.tensor.matmul(h1p[:], wt["w_e1h"][:, psl_],
                                         h_fr[c][:, lsl], start=True, stop=False)
                        nc.tensor.matmul(h1p[:], wt["w_cmb"][:, psl_],
                                         f_fr[c][:, lsl], start=False, stop=True)
                        h1 = grp.tile([128, GW], BF16, tag="h1", bufs=4)
                        nc.scalar.activation(h1[:], h1p[:], AF.Gelu,
                                             bias=wt["b_e1"][:, p:p + 1])
                        gb = ps.tile([128, GW], F32, tag="psB")
                        nc.tensor.matmul(gb[:], wt["sel"][:, psl_],
                                         g_fm[:, gsl], start=True, stop=True)
                        h1s = grp.tile([128, GW], BF16, tag="h1s", bufs=4)
                        nc.vector.tensor_tensor(h1s[:], h1[:], gb[:], ALU.mult)
                        nc.tensor.matmul(moe[:], wt["w_e2"][:, psl_], h1s[:],
                                         start=(p == 0), stop=False)
                    nc.tensor.matmul(moe[:], wt["b_e2"][:], g_fm[:, gsl],
                                     start=False, stop=True)
                    moe_sb = grp.tile([128, GW], BF16, tag="moe_sb", bufs=3)
                    nc.scalar.copy(moe_sb[:], moe[:])
                    moeT = grp.tile([128, GSZ, 128], BF16, tag="moeT", bufs=3)
                    nc.sync.dma_start_transpose(moeT[:], moe_sb[:])
                    nxt = grp.tile([128, GSZ, 128], F32, tag="nxt")
                    nc.vector.tensor_tensor(
                        nxt[:], hid_tm[:, GSZ * g:GSZ * (g + 1), 0:128],
                        moeT[:], ALU.add)
                    nc.sync.dma_start(next_od[:, GSZ * g:GSZ * (g + 1), :],
                                      nxt[:])

            # bf16 expert-input copies, double-buffered per chunk (parity)
            CW = CT * 128                 # tokens per chunk
            h_fr = {}
            f_fr = {}
            for c in range(NCH):
                h_fr[c] = frp.tile([128, CW], BF16, tag=f"h_fr{c % 2}", name=f"h_fr{c}")
                f_fr[c] = frp.tile([NSF, CW], BF16, tag=f"f_fr{c % 2}", name=f"f_fr{c}")

            # software pipeline: next chunk's transposes/router are emitted
            # before this chunk's gates/experts so PE never starves
            phase_A1(0)
            phase_router(0)
            phase_softmax(0)
            for c in range(NCH):
                if c + 1 < NCH:
                    phase_A1(c + 1)
                    phase_router(c + 1)
                phase_gfm(c)
                if c + 1 < NCH:
                    phase_softmax(c + 1)
                phase_experts(c)

    nc.compile()
    return nc


@functools.cache
def _built():
    return build_kernel()


def _fold_weights(inputs):
    """Host-side folding of all parameters into device weight tensors."""
    f32 = np.float32
    g = np.asarray(inputs["ln_g"], f32)
    b = np.asarray(inputs["ln_b"], f32)
    Wp = np.asarray(inputs["Wp"], f32)
    bp = np.asarray(inputs["bp"], f32)
    Wr1 = np.asarray(inputs["Wr1"], f32)
    br1 = np.asarray(inputs["br1"], f32)
    Wr2 = np.asarray(inputs["Wr2"], f32)
    br2 = np.asarray(inputs["br2"], f32)
    Wf = np.asarray(inputs["Wf"], f32)
    bf = np.asarray(inputs["bf"], f32)
    We1h = np.asarray(inputs["We1h"], f32)
    We1f = np.asarray(inputs["We1f"], f32)
    be1 = np.asarray(inputs["be1"], f32)
    We2 = np.asarray(inputs["We2"], f32)
    be2 = np.asarray(inputs["be2"], f32)
    alpha = float(np.asarray(inputs["alpha"], f32))
    expert_idx = np.asarray(inputs["expert_idx"]).astype(np.int64)
    stage_idx = np.asarray(inputs["stage_idx"]).astype(np.int64)

    Wr1h, Wr1f = Wr1[:DM], Wr1[DM:]
    # stage gather folded: femb = feat[:, stage_idx] @ Wp + bp
    Mstage = np.zeros((NSF, NSF), f32)
    np.add.at(Mstage, stage_idx, Wp)
    w_r1h = g[:, None] * Wr1h
    w_r1f = Mstage @ Wr1f
    b_r1 = br1 + b @ Wr1h + bp @ Wr1f
    w_r2a = np.zeros((2 * DRH, 2 * E), f32)
    w_r2a[:DRH, :E] = Wr2
    w_r2a[DRH:, E:] = Wr2
    br2exp = np.tile(np.exp(br2)[None, :], (128, 1)).astype(f32)

    # expert gather + Wf@We1f collapsed; pairs (2i, 2i+1) packed in columns
    w_cmb = np.zeros((NSF, NPAIR * 128), f32)
    w_e1h = np.zeros((DM, NPAIR * 128), f32)
    b_e1 = np.zeros((128, NPAIR), f32)
    for e in range(E):
        Se = np.zeros((NSF, NF), f32)
        Se[expert_idx[e], np.arange(NF)] = 1.0
        cmb = (Se @ Wf[e]) @ We1f[e]                    # [64, 64]
        p, half = e // 2, (e % 2) * DH
        w_cmb[:, 128 * p + half:128 * p + half + DH] = cmb
        w_e1h[:, 128 * p + half:128 * p + half + DH] = g[:, None] * We1h[e]
        b_e1[half:half + DH, p] = be1[e] + bf[e] @ We1f[e] + b @ We1h[e]

    # We2 stacked [E*DH, DM] * alpha; chunk c rows = experts 2c,2c+1
    w_e2 = np.zeros((128, NPAIR * 128), f32)
    W2s = (alpha * We2).reshape(E * DH, DM)
    for c in range(NPAIR):
        w_e2[:, 128 * c:128 * (c + 1)] = W2s[128 * c:128 * (c + 1)]
    b_e2 = (alpha * be2).astype(f32)

    sel = np.zeros((E, NPAIR * 128), f32)
    for p in range(NPAIR):
        sel[2 * p, 128 * p:128 * p + 64] = 1.0
        sel[2 * p + 1, 128 * p + 64:128 * (p + 1)] = 1.0
    ident = np.eye(128, dtype=f32)
    bf = ml_dtypes.bfloat16
    return dict(w_r1h=w_r1h, w_r1f=w_r1f, b_r1=np.tile(b_r1, 2)[:, None],
                w_r2a=w_r2a, br2exp=br2exp,
                w_cmb=w_cmb.astype(bf), w_e1h=w_e1h.astype(bf), b_e1=b_e1,
                w_e2=w_e2.astype(bf), b_e2=b_e2.astype(bf),
                sel=sel.astype(bf), ident=ident)


def make_in_maps(inputs):
    wts = _fold_weights(inputs)
    hidden = np.ascontiguousarray(np.asarray(inputs["hidden"], np.float32))
    feat = np.ascontiguousarray(np.asarray(inputs["feat"], np.float32))
    seq = np.asarray(inputs["item_seq_len"]).astype(np.int64)

    in_maps = []
    for c in range(NCORES):
        bs = slice(BC * c, BC * (c + 1))
        hid_c = hidden[bs].reshape(NTOK, DM)
        feat_c = feat[bs].reshape(NTOK, NSF)
        # token (n, p): flat = n*128 + p; b_local = n // (T//128); t = flat % T
        n_idx = np.arange(NT)
        p_idx = np.arange(128)
        flat = n_idx[None, :] * 128 + p_idx[:, None]      # [128, nt]
        tpos = flat % T
        bloc = flat // T
        valid = (tpos < seq[bs][bloc]).astype(np.float32)
        m = dict(hidden=hid_c, feat=feat_c, valid=valid)
        m.update(wts)
        in_maps.append(m)
    return in_maps


LAST_RESULT = None


def kernel(**inputs):
    global LAST_RESULT
    nc = _built()
    in_maps = make_in_maps(inputs)
    res = run_bass_kernel_spmd(nc, in_maps, core_ids=list(range(NCORES)))
    LAST_RESULT = res
    nh = np.stack([res.results[c]["next_h"].reshape(BC, T, DM)
                   for c in range(NCORES)]).reshape(B, T, DM)
    ga = np.stack([res.results[c]["gates_o"].reshape(BC, T, E)
                   for c in range(NCORES)]).reshape(B, T, E)
    pr = np.stack([res.results[c]["probs_o"].reshape(BC, T, E)
                   for c in range(NCORES)]).reshape(B, T, E)
    return nh, ga, pr


# revision 23
# speedup vs baseline: 2.0133x; 1.0635x over previous
"""Trainium2 Bass kernel for nn_MoEStage (MoE routing stage).

Data-parallel over batch B=32 across 8 NeuronCores (4 batches/core).
All params replicated and pre-folded on host:
  - LayerNorm affine (ln_g, ln_b) folded into Wr1/We1h weights+biases
  - stage_idx/expert_idx gathers folded into weight matrices
  - Wf @ We1f collapsed into a single [64, 64] per-expert matrix
  - alpha folded into We2/be2

Device layout: token-major tiles [128 tokens, D] for LN/softmax/top-k,
feature-major [D, tokens] (via PE transposes) for all matmuls.
"""
import sys
import functools

sys.path.insert(0, "/opt/trn_rl_repo")

import numpy as np
import ml_dtypes

import concourse.bacc as bacc
import concourse.mybir as mybir
import concourse.tile as tile
from concourse.bass import broadcast_tensor_aps
from concourse.bass_utils import run_bass_kernel_spmd

F32 = mybir.dt.float32
F32R = mybir.dt.float32r
BF16 = mybir.dt.bfloat16

def r32(ap):
    return ap.bitcast(F32R)

AF = mybir.ActivationFunctionType
ALU = mybir.AluOpType
AX = mybir.AxisListType

B, T, DM, DFE, DH, DRH = 32, 2048, 128, 64, 64, 64
E, NF, NSF, TOPK = 8, 16, 64, 2
LN_EPS = 1e-5
NCORES = 8
BC = B // NCORES          # batches per core
NTOK = BC * T             # 8192 tokens per core
NT = NTOK // 128          # 64 token-tiles of 128
GSZ = 4                   # tiles per group
NG = NT // GSZ            # 16 groups of 512 tokens
GW = GSZ * 128            # 512 tokens per group
NPAIR = 4                 # expert pairs


def build_kernel(ntok=NTOK):
    nt = ntok // 128
    ng = nt // GSZ
    nc = bacc.Bacc("TRN2", target_bir_lowering=False, debug=False,
                   num_devices=NCORES)

    din = {}
    R32_INPUTS = set()
    BF_INPUTS = {"w_cmb", "w_e1h", "w_e2", "b_e2", "sel"}
    def inp(name, shape):
        dt = BF16 if name in BF_INPUTS else F32
        din[name] = nc.dram_tensor(name, list(shape), dt, kind="ExternalInput")
        return din[name]

    hidden = inp("hidden", [ntok, DM])
    feat = inp("feat", [ntok, NSF])
    valid = inp("valid", [128, nt])
    w_r1h = inp("w_r1h", [DM, DRH])          # lhsT K=128 -> M=64
    w_r1f = inp("w_r1f", [NSF, DRH])         # lhsT K=64 -> M=64
    b_r1 = inp("b_r1", [2 * DRH, 1])   # duplicated for col-packed pairs
    w_r2a = inp("w_r2a", [2 * DRH, 2 * E])   # block-diag [Wr2 0; 0 Wr2]
    br2exp = inp("br2exp", [128, E])
    w_cmb = inp("w_cmb", [NSF, NPAIR * 128])   # per pair: [64, 128]
    w_e1h = inp("w_e1h", [DM, NPAIR * 128])    # per pair: [128, 128]
    b_e1 = inp("b_e1", [128, NPAIR])
    w_e2 = inp("w_e2", [128, NPAIR * 128])     # chunk c: [K=128, M=128]
    b_e2 = inp("b_e2", [E, DM])
    sel = inp("sel", [E, NPAIR * 128])
    ident = inp("ident", [128, 128])

    next_h = nc.dram_tensor("next_h", [ntok, DM], F32, kind="ExternalOutput")
    gates_o = nc.dram_tensor("gates_o", [ntok, E], F32, kind="ExternalOutput")
    probs_o = nc.dram_tensor("probs_o", [ntok, E], F32, kind="ExternalOutput")

    with tile.TileContext(nc) as tc:
        with (
            tc.tile_pool(name="wp", bufs=1) as wp,
            tc.tile_pool(name="big", bufs=1) as big,
            tc.tile_pool(name="wk", bufs=3) as wk,
            tc.tile_pool(name="frp", bufs=1) as frp,
            tc.tile_pool(name="grp", bufs=2) as grp,
            tc.tile_pool(name="ps", bufs=2, space="PSUM") as ps,
            tc.tile_pool(name="ps1", bufs=1, space="PSUM") as ps1,
        ):
            # ---- load weights/constants ----
            wt = {}
            for name, h in din.items():
                if name in ("hidden", "feat"):
                    continue
                t = wp.tile(list(h.shape), h.dtype, tag=f"w_{name}")
                nc.sync.dma_start(t[:], h[:])
                wt[name] = t

            # ---- load activations (token-major, hidden padded for bn_stats) ----
            hid_tm = big.tile([128, nt, 132], F32)
            nc.sync.dma_start(hid_tm[:, :, 0:128],
                              hidden.ap().rearrange("(n p) d -> p n d", p=128))
            feat_tm = big.tile([128, nt, NSF], F32)
            nc.sync.dma_start(feat_tm[:],
                              feat.ap().rearrange("(n p) d -> p n d", p=128))

            # ---- LayerNorm stats via per-tile bn_stats ----
            bn = big.tile([128, nt, 8], F32)
            for n in range(nt):
                nc.vector.bn_stats(bn[:, n, 0:6], hid_tm[:, n, 0:128])
            me, ve = bn[:, :, 1], bn[:, :, 2]
            mo, vo = bn[:, :, 4], bn[:, :, 5]
            mu = big.tile([128, nt], F32)
            nc.vector.tensor_tensor(mu[:], me, mo, ALU.add)
            nc.vector.tensor_scalar_mul(mu[:], mu[:], 0.5)
            dmn = big.tile([128, nt], F32)
            nc.vector.tensor_tensor(dmn[:], me, mo, ALU.subtract)
            nc.vector.tensor_tensor(dmn[:], dmn[:], dmn[:], ALU.mult)
            var = big.tile([128, nt], F32)
            nc.vector.tensor_tensor(var[:], ve, vo, ALU.add)
            nc.vector.scalar_tensor_tensor(var[:], var[:], 1.0 / DM, dmn[:],
                                           ALU.mult, ALU.bypass)
            nc.vector.scalar_tensor_tensor(var[:], dmn[:], 0.25, var[:],
                                           ALU.mult, ALU.add)
            rstd = big.tile([128, nt], F32)
            epsb = wp.tile([128, 1], F32, tag="epsb")
            nc.vector.memset(epsb[:], LN_EPS)
            nc.scalar.activation(rstd[:], var[:], AF.Ln, bias=epsb[:])
            nc.scalar.activation(rstd[:], rstd[:], AF.Exp, scale=-0.5)

            identb = wp.tile([128, 128], BF16, tag="identb")
            nc.gpsimd.tensor_copy(identb[:], wt["ident"][:])

            h_fm = big.tile([128, ntok], F32)
            f_fm = big.tile([NSF, ntok], F32)
            logit_sb = big.tile([128, nt, E], F32)
            Eexp = big.tile([128, nt, E], F32)
            probs = big.tile([128, nt, E], F32)
            msk = big.tile([128, nt, E], F32)
            G0 = big.tile([128, nt, E], F32)
            gates = big.tile([128, nt, E], F32)
            gates_bf = big.tile([128, nt, E], BF16)
            g_fm = big.tile([E, ntok], BF16)
            S = big.tile([128, nt], F32)
            Sr = big.tile([128, nt], F32)
            m1 = big.tile([128, nt], F32)
            m2 = big.tile([128, nt], F32)
            dsum = big.tile([128, nt], F32)
            dr = big.tile([128, nt], F32)

            def bcol(t, sl):  # [128, cw] -> broadcastable [128, cw, E]
                return t[:, sl].rearrange("p (n o) -> p n o", o=1)

            NCH = 4                       # chunks
            CG = ng // NCH                # groups per chunk (4)
            CT = CG * GSZ                 # tiles per chunk (16)
            probs_od = probs_o.ap().rearrange("(n p) e -> p n e", p=128)
            gates_od = gates_o.ap().rearrange("(n p) e -> p n e", p=128)
            next_od = next_h.ap().rearrange("(n p) d -> p n d", p=128)

            def phase_A1(c):
                # normalize + transpose to feature-major (+ bf16 expert copies)
                for g in range(CG * c, CG * (c + 1)):
                    hps = ps.tile([128, GW], F32, tag="psA")
                    fps = ps.tile([NSF, GW], F32, tag="psB")
                    for s in range(GSZ):
                        n = GSZ * g + s
                        xn = wk.tile([128, 128], F32, tag="xn", bufs=4)
                        nc.vector.tensor_scalar(xn[:], hid_tm[:, n, 0:128],
                                                mu[:, n:n + 1], rstd[:, n:n + 1],
                                                ALU.subtract, ALU.mult)
                        nc.tensor.transpose(hps[:, 128 * s:128 * (s + 1)],
                                            xn[:], wt["ident"][:])
                        nc.tensor.transpose(fps[:, 128 * s:128 * (s + 1)],
                                            feat_tm[:, n, :], wt["ident"][:])
                    gsl = slice(GW * g, GW * (g + 1))
                    nc.scalar.copy(h_fm[:, gsl], hps[:])
                    nc.scalar.copy(f_fm[:, gsl], fps[:])
                    lsl = slice(GW * g - CW * c, GW * (g + 1) - CW * c)
                    nc.vector.tensor_copy(h_fr[c][:, lsl], h_fm[:, gsl])
                    nc.vector.tensor_copy(f_fr[c][:, lsl], f_fm[:, gsl])

            def phase_router(c):
                for g2 in range(CG * c // 2, CG * (c + 1) // 2):
                    sl0 = slice(GW * 2 * g2, GW * (2 * g2 + 1))
                    sl1 = slice(GW * (2 * g2 + 1), GW * (2 * g2 + 2))
                    r1p = ps1.tile([128, GW], F32, tag="psC")
                    nc.tensor.matmul(r1p[0:DRH, :], wt["w_r1h"][:],
                                     h_fm[:, sl0], tile_position=(0, 0),
                                     start=True, stop=False,
                                     skip_group_check=True)
                    nc.tensor.matmul(r1p[DRH:128, :], wt["w_r1h"][:],
                                     h_fm[:, sl1], tile_position=(0, 64),
                                     start=True, stop=False,
                                     skip_group_check=True)
                    nc.tensor.matmul(r1p[0:DRH, :], wt["w_r1f"][:],
                                     f_fm[:, sl0], tile_position=(0, 0),
                                     start=False, stop=True,
                                     skip_group_check=True)
                    nc.tensor.matmul(r1p[DRH:128, :], wt["w_r1f"][:],
                                     f_fm[:, sl1], tile_position=(0, 64),
                                     start=False, stop=True,
                                     skip_group_check=True)
                    r1sb = grp.tile([128, GW], F32, tag="r1sb")
                    nc.scalar.activation(r1sb[:], r1p[:], AF.Relu,
                                         bias=wt["b_r1"][:])
                    logit_ps = ps1.tile([128, GSZ, 2, E], F32, tag="psD", bufs=2)
                    for s in range(GSZ):
                        nc.tensor.matmul(logit_ps[:, s, :, :],
                                         r1sb[:, 128 * s:128 * (s + 1)],
                                         wt["w_r2a"][:], start=True, stop=True)
                    dst = logit_sb[:, GSZ * 2 * g2:GSZ * (2 * g2 + 2), :]
                    nc.scalar.copy(dst.rearrange("p (h s) e -> p s h e", h=2),
                                   logit_ps[:])

            def phase_softmax(c):
                tsl = slice(CT * c, CT * (c + 1))
                nc.scalar.activation(Eexp[:, tsl, :], logit_sb[:, tsl, :],
                                     AF.Exp)
                a, b_ = broadcast_tensor_aps(
                    Eexp[:, tsl, :],
                    wt["br2exp"][:].rearrange("p (n e) -> p n e", n=1))
                nc.vector.tensor_tensor(Eexp[:, tsl, :], a, b_, ALU.mult)
                nc.vector.reduce_sum(S[:, tsl], Eexp[:, tsl, :], axis=AX.X)
                nc.vector.reciprocal(Sr[:, tsl], S[:, tsl])
                a, b_ = broadcast_tensor_aps(Eexp[:, tsl, :], bcol(Sr, tsl))
                nc.vector.tensor_tensor(probs[:, tsl, :], a, b_, ALU.mult)
                nc.sync.dma_start(probs_od[:, tsl, :], probs[:, tsl, :])
                nc.vector.reduce_max(m1[:, tsl], Eexp[:, tsl, :], axis=AX.X)
                a, b_ = broadcast_tensor_aps(Eexp[:, tsl, :], bcol(m1, tsl))
                nc.vector.tensor_tensor(msk[:, tsl, :], a, b_, ALU.is_lt)
                nc.vector.tensor_tensor(G0[:, tsl, :], Eexp[:, tsl, :],
                                        msk[:, tsl, :], ALU.mult)
                nc.vector.reduce_max(m2[:, tsl], G0[:, tsl, :], axis=AX.X)
                a, b_ = broadcast_tensor_aps(Eexp[:, tsl, :], bcol(m2, tsl))
                nc.vector.tensor_tensor(msk[:, tsl, :], a, b_, ALU.is_ge)
                nc.vector.tensor_tensor(G0[:, tsl, :], Eexp[:, tsl, :],
                                        msk[:, tsl, :], ALU.mult)
                nc.vector.reduce_sum(dsum[:, tsl], G0[:, tsl, :], axis=AX.X)
                nc.vector.reciprocal(dr[:, tsl], dsum[:, tsl])
                nc.vector.tensor_tensor(dr[:, tsl], dr[:, tsl],
                                        wt["valid"][:, tsl], ALU.mult)
                a, b_ = broadcast_tensor_aps(G0[:, tsl, :], bcol(dr, tsl))
                nc.vector.tensor_tensor(gates[:, tsl, :], a, b_, ALU.mult)
                nc.sync.dma_start(gates_od[:, tsl, :], gates[:, tsl, :])
                nc.vector.tensor_copy(gates_bf[:, tsl, :], gates[:, tsl, :])

            def phase_gfm(c):
                for g in range(CG * c, CG * (c + 1)):
                    gps = ps1.tile([E, GW], BF16, tag="psD", bufs=2)
                    for s in range(GSZ):
                        n = GSZ * g + s
                        nc.tensor.transpose(gps[:, 128 * s:128 * (s + 1)],
                                            gates_bf[:, n, :], identb[:])
                    nc.scalar.copy(g_fm[:, GW * g:GW * (g + 1)], gps[:])

            def phase_experts(c):
                for g in range(CG * c, CG * (c + 1)):
                    gsl = slice(GW * g, GW * (g + 1))
                    moe = ps1.tile([128, GW], F32, tag="psCm")
                    for p in range(NPAIR):
                        psl_ = slice(128 * p, 128 * (p + 1))
                        h1p = ps.tile([128, GW], F32, tag="psA")
                        lsl = slice(GW * g - CW * c, GW * (g + 1) - CW * c)
                        nc.tensor.matmul(h1p[:], wt["w_e1h"][:, psl_],
                                         h_fr[c][:, lsl], start=True, stop=False)
                        nc.tensor.matmul(h1p[:], wt["w_cmb"][:, psl_],
                                         f_fr[c][:, lsl], start=False, stop=True)
                        h1 = grp.tile([128, GW], BF16, tag="h1", bufs=4)
                        nc.scalar.activation(h1[:], h1p[:], AF.Gelu,
                                             bias=wt["b_e1"][:, p:p + 1])
                        gb = ps.tile([128, GW], F32, tag="psB")
                        nc.tensor.matmul(gb[:], wt["sel"][:, psl_],
                                         g_fm[:, gsl], start=True, stop=True)
                        h1s = grp.tile([128, GW], BF16, tag="h1s", bufs=4)
                        nc.vector.tensor_tensor(h1s[:], h1[:], gb[:], ALU.mult)
                        nc.tensor.matmul(moe[:], wt["w_e2"][:, psl_], h1s[:],
                                         start=(p == 0), stop=False)
                    nc.tensor.matmul(moe[:], wt["b_e2"][:], g_fm[:, gsl],
                                     start=False, stop=True)
                    moe_sb = grp.tile([128, GW], BF16, tag="moe_sb", bufs=3)
                    nc.scalar.copy(moe_sb[:], moe[:])
                    moeT = grp.tile([128, GSZ, 128], BF16, tag="moeT", bufs=3)
                    nc.sync.dma_start_transpose(moeT[:], moe_sb[:])
                    nxt = grp.tile([128, GSZ, 128], F32, tag="nxt")
                    nc.vector.tensor_tensor(
                        nxt[:], hid_tm[:, GSZ * g:GSZ * (g + 1), 0:128],
                        moeT[:], ALU.add)
                    nc.sync.dma_start(next_od[:, GSZ * g:GSZ * (g + 1), :],
                                      nxt[:])

            # bf16 expert-input copies, double-buffered per chunk (parity)
            CW = CT * 128                 # tokens per chunk
            h_fr = {}
            f_fr = {}
            for c in range(NCH):
                h_fr[c] = frp.tile([128, CW], BF16, tag=f"h_fr{c % 2}", name=f"h_fr{c}")
                f_fr[c] = frp.tile([NSF, CW], BF16, tag=f"f_fr{c % 2}", name=f"f_fr{c}")

            # software pipeline: next chunk's transposes/router are emitted
            # before this chunk's gates/experts so PE never starves
            phase_A1(0)
            phase_router(0)
            phase_softmax(0)
            for c in range(NCH):
                if c + 1 < NCH:
                    phase_A1(c + 1)
                    phase_router(c + 1)
                phase_gfm(c)
                if c + 1 < NCH:
                    phase_softmax(c + 1)
                phase_experts(c)

    nc.compile()
    return nc


@functools.cache
def _built():
    return build_kernel()


def _fold_weights(inputs):
    """Host-side folding of all parameters into device weight tensors."""
    f32 = np.float32
    g = np.asarray(inputs["ln_g"], f32)
    b = np.asarray(inputs["ln_b"], f32)
    Wp = np.asarray(inputs["Wp"], f32)
    bp = np.asarray(inputs["bp"], f32)
    Wr1 = np.asarray(inputs["Wr1"], f32)
    br1 = np.asarray(inputs["br1"], f32)
    Wr2 = np.asarray(inputs["Wr2"], f32)
    br2 = np.asarray(inputs["br2"], f32)
    Wf = np.asarray(inputs["Wf"], f32)
    bf = np.asarray(inputs["bf"], f32)
    We1h = np.asarray(inputs["We1h"], f32)
    We1f = np.asarray(inputs["We1f"], f32)
    be1 = np.asarray(inputs["be1"], f32)
    We2 = np.asarray(inputs["We2"], f32)
    be2 = np.asarray(inputs["be2"], f32)
    alpha = float(np.asarray(inputs["alpha"], f32))
    expert_idx = np.asarray(inputs["expert_idx"]).astype(np.int64)
    stage_idx = np.asarray(inputs["stage_idx"]).astype(np.int64)

    Wr1h, Wr1f = Wr1[:DM], Wr1[DM:]
    # stage gather folded: femb = feat[:, stage_idx] @ Wp + bp
    Mstage = np.zeros((NSF, NSF), f32)
    np.add.at(Mstage, stage_idx, Wp)
    w_r1h = g[:, None] * Wr1h
    w_r1f = Mstage @ Wr1f
    b_r1 = br1 + b @ Wr1h + bp @ Wr1f
    w_r2a = np.zeros((2 * DRH, 2 * E), f32)
    w_r2a[:DRH, :E] = Wr2
    w_r2a[DRH:, E:] = Wr2
    br2exp = np.tile(np.exp(br2)[None, :], (128, 1)).astype(f32)

    # expert gather + Wf@We1f collapsed; pairs (2i, 2i+1) packed in columns
    w_cmb = np.zeros((NSF, NPAIR * 128), f32)
    w_e1h = np.zeros((DM, NPAIR * 128), f32)
    b_e1 = np.zeros((128, NPAIR), f32)
    for e in range(E):
        Se = np.zeros((NSF, NF), f32)
        Se[expert_idx[e], np.arange(NF)] = 1.0
        cmb = (Se @ Wf[e]) @ We1f[e]                    # [64, 64]
        p, half = e // 2, (e % 2) * DH
        w_cmb[:, 128 * p + half:128 * p + half + DH] = cmb
        w_e1h[:, 128 * p + half:128 * p + half + DH] = g[:, None] * We1h[e]
        b_e1[half:half + DH, p] = be1[e] + bf[e] @ We1f[e] + b @ We1h[e]

    # We2 stacked [E*DH, DM] * alpha; chunk c rows = experts 2c,2c+1
    w_e2 = np.zeros((128, NPAIR * 128), f32)
    W2s = (alpha * We2).reshape(E * DH, DM)
    for c in range(NPAIR):
        w_e2[:, 128 * c:128 * (c + 1)] = W2s[128 * c:128 * (c + 1)]
    b_e2 = (alpha * be2).astype(f32)

    sel = np.zeros((E, NPAIR * 128), f32)
    for p in range(NPAIR):
        sel[2 * p, 128 * p:128 * p + 64] = 1.0
        sel[2 * p + 1, 128 * p + 64:128 * (p + 1)] = 1.0
    ident = np.eye(128, dtype=f32)
    bf = ml_dtypes.bfloat16
    return dict(w_r1h=w_r1h, w_r1f=w_r1f, b_r1=np.tile(b_r1, 2)[:, None],
                w_r2a=w_r2a, br2exp=br2exp,
                w_cmb=w_cmb.astype(bf), w_e1h=w_e1h.astype(bf), b_e1=b_e1,
                w_e2=w_e2.astype(bf), b_e2=b_e2.astype(bf),
                sel=sel.astype(bf), ident=ident)


def make_in_maps(inputs):
    wts = _fold_weights(inputs)
    hidden = np.ascontiguousarray(np.asarray(inputs["hidden"], np.float32))
    feat = np.ascontiguousarray(np.asarray(inputs["feat"], np.float32))
    seq = np.asarray(inputs["item_seq_len"]).astype(np.int64)

    in_maps = []
    for c in range(NCORES):
        bs = slice(BC * c, BC * (c + 1))
        hid_c = hidden[bs].reshape(NTOK, DM)
        feat_c = feat[bs].reshape(NTOK, NSF)
        # token (n, p): flat = n*128 + p; b_local = n // (T//128); t = flat % T
        n_idx = np.arange(NT)
        p_idx = np.arange(128)
        flat = n_idx[None, :] * 128 + p_idx[:, None]      # [128, nt]
        tpos = flat % T
        bloc = flat // T
        valid = (tpos < seq[bs][bloc]).astype(np.float32)
        m = dict(hidden=hid_c, feat=feat_c, valid=valid)
        m.update(wts)
        in_maps.append(m)
    return in_maps


LAST_RESULT = None


def kernel(**inputs):
    global LAST_RESULT
    nc = _built()
    in_maps = make_in_maps(inputs)
    res = run_bass_kernel_spmd(nc, in_maps, core_ids=list(range(NCORES)))
    LAST_RESULT = res
    nh = np.stack([res.results[c]["next_h"].reshape(BC, T, DM)
                   for c in range(NCORES)]).reshape(B, T, DM)
    ga = np.stack([res.results[c]["gates_o"].reshape(BC, T, E)
                   for c in range(NCORES)]).reshape(B, T, E)
    pr = np.stack([res.results[c]["probs_o"].reshape(BC, T, E)
                   for c in range(NCORES)]).reshape(B, T, E)
    return nh, ga, pr
